# revision 25
# baseline (speedup 1.0000x reference)
"""Multi-head graph attention (GAT) Trainium2 kernel, v2.

Row-sharded across 8 NeuronCores: core i owns queries [i*1024, (i+1)*1024).

Math (per head h, with Wh = h @ W_h, a = Wh@a1, b = Wh@a2, s = a_i + b_j):
    e[i,j]  = leakyrelu(s, 0.2)
    attn    = softmax_j(where(adj>0, e, -9e15))
    out_h   = elu(attn @ Wh)
    out     = concat_h(out_h) @ Wp.T + bp

On-chip factorization (exact): exp(lrelu(s)) = exp(0.2s) * max(exp(0.8s), 1).
The per-query factor exp(0.2 a_i) cancels in softmax, so the unnormalized
weight used on-chip is
    w[j,i] = adjT[j,i] * vb02_j * max(exp(0.8 a_i + 0.8 b_j), 1)
with vb02_j = exp(0.2 b_j) folded into the value stationaries host-side.

Per key-block (128 keys x 1024 queries), per head the masked weights are
built one of two ways (to spread work across engines):
  ACT-form (heads 0,1):  e = ScalarE exp(abc + b08_j)  [per-partition bias],
                         pm = DVE stt: (e max 1) * mask
  z-form  (heads 2,3):   z = ea08b * mask   (TT mult, DVE/GPSIMD)
                         pm = (z * vb08_j) max mask    (stt, DVE/GPSIMD)
      since mask in {0,1}: max(z*vb08, mask) = mask * max(exp(.8s), 1).

All setup tensors (Wh, score rows, exp factors, scaled stationaries) are
precomputed on host; device setup is pure DMA + 8 small broadcast matmuls.
adj is host-transposed to bf16 so mask loads are plain contiguous DMAs.
"""

import os
from contextlib import ExitStack

import numpy as np
import ml_dtypes

import concourse.bacc as bacc
import concourse.bass as bass
import concourse.mybir as mybir
import concourse.tile as tile

F32 = mybir.dt.float32
BF16 = mybir.dt.bfloat16

ALU = mybir.AluOpType
AF = mybir.ActivationFunctionType

N = 8192          # nodes
IN_F = 256        # input features
H = 4             # heads
DH = 64           # head dim
NCORES = 8
QN = N // NCORES  # queries per core (1024)
KB = N // 128     # key blocks of 128 (64)
QH = 2            # 512-wide query halves

BF16_NP = ml_dtypes.bfloat16


def build_nc():
    nc = bacc.Bacc("TRN2", target_bir_lowering=False, debug=False)

    # host-precomputed tensors
    whv_d = nc.declare_dram_parameter("whv", [128, KB * H * (DH + 1)], BF16, False)
    adjt_d = nc.declare_dram_parameter("adjt", [N, QN], BF16, False)
    # heads 2,3: mask pre-scaled by vb08 = exp(0.8 b_j) host-side
    adjm_d = nc.declare_dram_parameter("adjm", [N, 2 * QN], BF16, False)
    a08_d = nc.declare_dram_parameter("a08", [2, QN], F32, False)      # heads 0,1: 0.8*a
    ea08_d = nc.declare_dram_parameter("ea08", [2, QN], BF16, False)   # heads 2,3: exp(0.8*a)
    b08_d = nc.declare_dram_parameter("b08", [128, 2 * KB], F32, False)    # heads 0,1
    wpt_d = nc.declare_dram_parameter("wpt", [IN_F, IN_F], F32, False)  # Wp.T
    bp_d = nc.declare_dram_parameter("bp", [IN_F], F32, False)
    out = nc.declare_dram_parameter("out", [QN, IN_F], F32, True)

    with ExitStack() as ctx:
        tc = ctx.enter_context(tile.TileContext(nc))

        persist = ctx.enter_context(tc.tile_pool(name="persist", bufs=1))
        whv = persist.tile([128, KB, H, DH + 1], BF16)
        abc = persist.tile([128, 2, QN], F32)      # broadcast 0.8*a rows, heads 0,1
        eap23 = persist.tile([128, 2, QN], BF16)   # broadcast exp(0.8a), heads 2,3
        b08 = persist.tile([128, 2, KB], F32)
        wpt_sb = persist.tile([128, 2, IN_F], F32)
        bpb = persist.tile([128, IN_F], F32)
        ones_b = persist.tile([1, 128], BF16)
        ones_f32 = persist.tile([1, 128], F32)
        ones_f = persist.tile([1, 64], F32)

        # main-loop pools pinned before setup so slots don't alias setup tiles
        MBUFS = int(os.environ.get("GAT_BUFS", "4"))
        mloop = ctx.enter_context(tc.tile_pool(name="mloop", bufs=MBUFS))
        for _b in range(MBUFS):
            _t = mloop.tile([128, QN], BF16, tag="mt")
            nc.vector.memset(_t[0:1, 0:2], 0.0)
            _t = mloop.tile([128, 2, QN], BF16, tag="mp23")
            nc.vector.memset(_t[0:1, 0, 0:2], 0.0)
            _t = mloop.tile([128, 2, QN], BF16, tag="ee")
            nc.vector.memset(_t[0:1, 0, 0:2], 0.0)
            _t = mloop.tile([128, 2, QN], BF16, tag="q01")
            nc.vector.memset(_t[0:1, 0, 0:2], 0.0)
            _t = mloop.tile([128, 2, QN], BF16, tag="q23")
            nc.vector.memset(_t[0:1, 0, 0:2], 0.0)
            _t = mloop.tile([128, 2, QN], BF16, tag="pm01")
            nc.vector.memset(_t[0:1, 0, 0:2], 0.0)
            _t = mloop.tile([128, 2, QN], BF16, tag="pm23")
            nc.vector.memset(_t[0:1, 0, 0:2], 0.0)

        # ---------------- setup: DMAs + row broadcasts ----------------
        nc.vector.memset(ones_b, 1.0)
        nc.vector.memset(ones_f32, 1.0)
        nc.vector.memset(ones_f, 1.0)

        nc.scalar.dma_start(whv, whv_d[:, :].rearrange("p (k h d) -> p k h d", k=KB, h=H))
        nc.scalar.dma_start(b08, b08_d[:, :].rearrange("p (j k) -> p j k", j=2))
        nc.scalar.dma_start(wpt_sb, wpt_d[:, :].rearrange("(c p) w -> p c w", p=128))
        bp_ap = bp_d[:]
        nc.gpsimd.dma_start(bpb, bass.AP(tensor=bp_ap.tensor, offset=bp_ap.offset,
                                         ap=[[0, 128]] + list(bp_ap.ap)))

        WARMUP = int(os.environ.get("GAT_WARMUP", "24"))
        with tc.tile_pool(name="setup", bufs=1) as setup, \
             tc.tile_pool(name="spsum", bufs=4, space="PSUM") as spsum:
            a08row = setup.tile([1, 2, QN], F32)
            ea08row = setup.tile([1, 2, QN], BF16)
            nc.scalar.dma_start(a08row, a08_d[:, :].rearrange("(o j) q -> o j q", o=1))
            nc.scalar.dma_start(ea08row, ea08_d[:, :].rearrange("(o j) q -> o j q", o=1))
            # broadcast rows across 128 partitions via ones-matmuls
            for j in range(2):
                for qh in range(QH):
                    qsl = slice(qh * 512, (qh + 1) * 512)
                    pa = spsum.tile([128, 512], F32, tag="bc_a")
                    nc.tensor.matmul(pa, ones_f32, a08row[:, j, qsl])
                    nc.vector.tensor_copy(abc[:, j, qsl], pa)
                    pe = spsum.tile([128, 512], F32, tag="bc_e")
                    nc.tensor.matmul(pe, ones_b, ea08row[:, j, qsl])
                    nc.scalar.copy(eap23[:, j, qsl], pe)
            # PE warm-up: back-to-back dummy matmuls to flip HAM to 8/8
            # before the main accumulation begins
            for w in range(WARMUP):
                pw = spsum.tile([128, 512], F32, tag="bc_a")
                nc.tensor.matmul(pw[0:DH + 1, :], whv[:, 0, 0, :], eap23[:, 0, 0:512])

        # ---------------- main loop ----------------
        mpsum_cm = tc.tile_pool(name="mpsum", bufs=1, space="PSUM")
        mpsum = mpsum_cm.__enter__()
        acc = mpsum.tile([DH + 1, H, QH, 512], F32)

        # engine split: of the 128 mult pair-TT ops (2/block), TT_GPS go to
        # GPSIMD (Pool rejects max-TT), the rest to DVE.
        TT_GPS = int(os.environ.get("GAT_TT_GPS", "83"))  # per 128

        mi = 0

        def frac_hit(i, frac, tot):
            return (i * frac) // tot != ((i - 1) * frac) // tot

        def tt_engine():
            nonlocal mi
            mi += 1
            return nc.gpsimd if frac_hit(mi, TT_GPS, 128) else nc.vector

        for kb in range(KB):
            mt = mloop.tile([128, QN], BF16, tag="mt")
            nc.sync.dma_start(mt, adjt_d[kb * 128:(kb + 1) * 128, :])
            mt2 = bass.AP(tensor=mt.tensor, offset=mt.offset,
                          ap=[list(mt.ap[0]), [0, 2], list(mt.ap[1])])
            mp23 = mloop.tile([128, 2, QN], BF16, tag="mp23")
            nc.sync.dma_start(
                mp23, adjm_d[kb * 128:(kb + 1) * 128, :].rearrange(
                    "p (j q) -> p j q", j=2))

            # heads 0,1: ACT exp (per-partition bias); mask via
            # pm = max(e*mt, mt)  [= mt * max(e, 1) since mt in {0,1}]
            ee = mloop.tile([128, 2, QN], BF16, tag="ee")
            for j in range(2):
                nc.scalar.activation(ee[:, j, :], abc[:, j, :], AF.Exp,
                                     bias=b08[:, j, kb:kb + 1], scale=1.0)
            q01 = mloop.tile([128, 2, QN], BF16, tag="q01")
            tt_engine().tensor_tensor(q01, ee, mt2, op=ALU.mult)
            pm01 = mloop.tile([128, 2, QN], BF16, tag="pm01")
            nc.vector.tensor_tensor(pm01, q01, mt2, op=ALU.max)

            # heads 2,3: vb08 pre-folded into adjm; pm = max(ea * madj, mt)
            q23 = mloop.tile([128, 2, QN], BF16, tag="q23")
            tt_engine().tensor_tensor(q23, eap23, mp23, op=ALU.mult)
            pm23 = mloop.tile([128, 2, QN], BF16, tag="pm23")
            nc.vector.tensor_tensor(pm23, q23, mt2, op=ALU.max)

            for hs in range(H):
                pm = pm01 if hs < 2 else pm23
                j = hs % 2
                for qh in range(QH):
                    nc.tensor.matmul(acc[:, hs, qh, :], whv[:, kb, hs, :],
                                     pm[:, j, qh * 512:(qh + 1) * 512],
                                     start=(kb == 0), stop=(kb == KB - 1))

        # ---------------- tail: normalize, elu, out-proj ----------------
        tailp = ctx.enter_context(tc.tile_pool(name="tailp", bufs=1))
        denr = tailp.tile([1, H, QN], F32)
        gfin = tailp.tile([128, 2, QN], F32)
        graw = tailp.tile([128, 2, QN], F32)
        ACT_RECIP = int(os.environ.get("GAT_ACT_RECIP", "1"))
        for hs in range(H):
            for qh in range(QH):
                qsl = slice(qh * 512, (qh + 1) * 512)
                if ACT_RECIP:
                    # 1/den = square(1/sqrt(den)) on ACT (den > 0), keeping
                    # the iterative-divide off the DVE critical path
                    nc.scalar.activation(denr[:, hs, qsl], acc[DH:DH + 1, hs, qh, :],
                                         AF.Abs_reciprocal_sqrt)
                else:
                    nc.vector.reciprocal(denr[:, hs, qsl], acc[DH:DH + 1, hs, qh, :])
            graw_dst = graw[(hs % 2) * 64:(hs % 2) * 64 + 64, hs // 2, :]
            graw_src = acc[0:DH, hs, :, :].rearrange("p a b -> p (a b)")
            if hs % 2 == 0:
                nc.scalar.copy(graw_dst, graw_src)
            else:
                nc.vector.tensor_copy(graw_dst, graw_src)
        if ACT_RECIP:
            nc.scalar.activation(denr[0:1, :, :], denr[0:1, :, :], AF.Square)
        mpsum_cm.__exit__(None, None, None)

        with tc.tile_pool(name="tpsum", bufs=2, space="PSUM") as tpsum:
            # normalize: broadcast 1/den across partitions via ones-matmul
            for j in range(2):
                for qh in range(QH):
                    qsl = slice(qh * 512, (qh + 1) * 512)
                    rps = tpsum.tile([128, 512], F32, tag="r_ps")
                    nc.tensor.matmul(rps[0:64, :], ones_f, denr[:, 2 * j, qsl])
                    nc.tensor.matmul(rps[64:128, :], ones_f, denr[:, 2 * j + 1, qsl])
                    nc.vector.tensor_mul(gfin[:, j, qsl], graw[:, j, qsl], rps)

            # elu(x) = relu(x) + exp(min(x, 0)) - 1
            for j in range(2):
                for qh in range(QH):
                    qsl = slice(qh * 512, (qh + 1) * 512)
                    t = tailp.tile([128, 512], F32, tag="elu_t")
                    nc.vector.tensor_scalar(t, gfin[:, j, qsl], 0.0, None,
                                            op0=ALU.min)
                    e = tailp.tile([128, 512], F32, tag="elu_e")
                    nc.scalar.activation(e, t, AF.Exp)
                    em1 = tailp.tile([128, 512], F32, tag="elu_em1")
                    nc.vector.tensor_scalar(em1, e, -1.0, None, op0=ALU.add)
                    nc.vector.scalar_tensor_tensor(gfin[:, j, qsl], gfin[:, j, qsl],
                                                   0.0, em1, op0=ALU.max, op1=ALU.add)

            for qc in range(QN // 128):
                qsl = slice(qc * 128, (qc + 1) * 128)
                po = tpsum.tile([128, IN_F], F32, tag="out_ps")
                nc.tensor.matmul(po, gfin[:, 0, qsl], wpt_sb[:, 0, :],
                                 start=True, stop=False)
                nc.tensor.matmul(po, gfin[:, 1, qsl], wpt_sb[:, 1, :],
                                 start=False, stop=True)
                fin = tailp.tile([128, IN_F], F32, tag="fin")
                nc.vector.scalar_tensor_tensor(fin, po, 0.0, bpb,
                                               op0=ALU.add, op1=ALU.add)
                nc.sync.dma_start(out[qsl, :], fin)

    nc.compile()
    return nc


_NC_CACHE = {}
LAST_RESULTS = None


def _get_nc():
    if "nc" not in _NC_CACHE:
        _NC_CACHE["nc"] = build_nc()
    return _NC_CACHE["nc"]


def kernel(h, adj, W, a1, a2, Wp, bp):
    from concourse.bass_utils import run_bass_kernel_spmd

    h = np.asarray(h, dtype=np.float32)
    adj = np.asarray(adj)
    W = np.asarray(W, dtype=np.float32)
    a1 = np.asarray(a1, dtype=np.float32)
    a2 = np.asarray(a2, dtype=np.float32)
    Wp = np.asarray(Wp, dtype=np.float32)
    bp = np.asarray(bp, dtype=np.float32)

    # ---- host precompute (O(N d^2): ~1% of kernel FLOPs) ----
    Wh = np.einsum("ni,hid->nhd", h, W).astype(np.float32)     # [N, H, DH]
    asc = np.einsum("nhd,hd->hn", Wh, a1)                      # [H, N]
    bsc = np.einsum("nhd,hd->hn", Wh, a2)                      # [H, N]
    vb02 = np.exp(0.2 * bsc)                                   # [H, N]
    vb08 = np.exp(0.8 * bsc)
    # value stationaries [128, KB, H, DH+1]: [Wh * vb02 | vb02]
    whv_f = np.concatenate(
        [Wh * vb02.T[:, :, None], vb02.T[:, :, None]], axis=2)  # [N, H, DH+1]
    whv_np = np.ascontiguousarray(
        whv_f.reshape(KB, 128, H, DH + 1).transpose(1, 0, 2, 3)
        .reshape(128, KB * H * (DH + 1)).astype(BF16_NP))
    b08_np = np.ascontiguousarray(
        (0.8 * bsc[0:2]).T.reshape(KB, 128, 2).transpose(1, 2, 0)
        .reshape(128, 2 * KB).astype(np.float32))
    wpt = np.ascontiguousarray(Wp.T)

    nc = _get_nc()
    in_maps = []
    for c in range(NCORES):
        qsl = slice(c * QN, (c + 1) * QN)
        adjt_f = adj[qsl, :].T.astype(np.float32)           # [N, QN]
        adjm = adjt_f[:, None, :] * vb08[2:4].T[:, :, None]  # [N, 2, QN]
        in_maps.append({
            "whv": whv_np,
            "adjt": adjt_f.astype(BF16_NP),
            "adjm": adjm.reshape(N, 2 * QN).astype(BF16_NP),
            "a08": np.ascontiguousarray(0.8 * asc[0:2, qsl]).astype(np.float32),
            "ea08": np.ascontiguousarray(np.exp(0.8 * asc[2:4, qsl])).astype(BF16_NP),
            "b08": b08_np,
            "wpt": wpt,
            "bp": bp,
        })

    res = run_bass_kernel_spmd(nc, in_maps, core_ids=list(range(NCORES)))
    global LAST_RESULTS
    LAST_RESULTS = res
    return np.concatenate([r["out"] for r in res.results], axis=0)


# revision 26
# speedup vs baseline: 1.1648x; 1.1648x over previous
"""Multi-head graph attention (GAT) Trainium2 kernel, v2.

Row-sharded across 8 NeuronCores: core i owns queries [i*1024, (i+1)*1024).

Math (per head h, with Wh = h @ W_h, a = Wh@a1, b = Wh@a2, s = a_i + b_j):
    e[i,j]  = leakyrelu(s, 0.2)
    attn    = softmax_j(where(adj>0, e, -9e15))
    out_h   = elu(attn @ Wh)
    out     = concat_h(out_h) @ Wp.T + bp

On-chip factorization (exact): exp(lrelu(s)) = exp(0.2s) * max(exp(0.8s), 1).
The per-query factor exp(0.2 a_i) cancels in softmax, so the unnormalized
weight used on-chip is
    w[j,i] = adjT[j,i] * vb02_j * max(exp(0.8 a_i + 0.8 b_j), 1)
with vb02_j = exp(0.2 b_j) folded into the value stationaries host-side.

Per key-block (128 keys x 1024 queries), per head the masked weights are
built one of two ways (to spread work across engines):
  ACT-form (heads 0,1):  e = ScalarE exp(abc + b08_j)  [per-partition bias],
                         pm = DVE stt: (e max 1) * mask
  z-form  (heads 2,3):   z = ea08b * mask   (TT mult, DVE/GPSIMD)
                         pm = (z * vb08_j) max mask    (stt, DVE/GPSIMD)
      since mask in {0,1}: max(z*vb08, mask) = mask * max(exp(.8s), 1).

All setup tensors (Wh, score rows, exp factors, scaled stationaries) are
precomputed on host; device setup is pure DMA + 8 small broadcast matmuls.
adj is host-transposed to bf16 so mask loads are plain contiguous DMAs.
"""

import os
from contextlib import ExitStack

import numpy as np
import ml_dtypes

import concourse.bacc as bacc
import concourse.bass as bass
import concourse.mybir as mybir
import concourse.tile as tile

F32 = mybir.dt.float32
BF16 = mybir.dt.bfloat16

ALU = mybir.AluOpType
AF = mybir.ActivationFunctionType

N = 8192          # nodes
IN_F = 256        # input features
H = 4             # heads
DH = 64           # head dim
NCORES = 8
QN = N // NCORES  # queries per core (1024)
KB = N // 128     # key blocks of 128 (64)
QH = 2            # 512-wide query halves

BF16_NP = ml_dtypes.bfloat16


def build_nc():
    nc = bacc.Bacc("TRN2", target_bir_lowering=False, debug=False)

    # host-precomputed tensors
    whv_d = nc.declare_dram_parameter("whv", [128, KB * H * (DH + 1)], BF16, False)
    adjt_d = nc.declare_dram_parameter("adjt", [N, QN], BF16, False)
    # heads 2,3: mask pre-scaled by vb08 = exp(0.8 b_j) host-side
    adjm_d = nc.declare_dram_parameter("adjm", [N, 2 * QN], BF16, False)
    a08_d = nc.declare_dram_parameter("a08", [2, QN], F32, False)      # heads 0,1: 0.8*a
    ea08_d = nc.declare_dram_parameter("ea08", [2, QN], BF16, False)   # heads 2,3: exp(0.8*a)
    b08_d = nc.declare_dram_parameter("b08", [128, 2 * KB], F32, False)    # heads 0,1
    wpt_d = nc.declare_dram_parameter("wpt", [IN_F, IN_F], F32, False)  # Wp.T
    bp_d = nc.declare_dram_parameter("bp", [IN_F], F32, False)
    out = nc.declare_dram_parameter("out", [QN, IN_F], F32, True)

    with ExitStack() as ctx:
        tc = ctx.enter_context(tile.TileContext(nc))

        persist = ctx.enter_context(tc.tile_pool(name="persist", bufs=1))
        whv = persist.tile([128, KB, H, DH + 1], BF16)
        abc = persist.tile([128, 2, QN], F32)      # broadcast 0.8*a rows, heads 0,1
        eap23 = persist.tile([128, 2, QN], BF16)   # broadcast exp(0.8a), heads 2,3
        b08 = persist.tile([128, 2, KB], F32)
        wpt_sb = persist.tile([128, 2, IN_F], F32)
        bpb = persist.tile([128, IN_F], F32)
        ones_b = persist.tile([1, 128], BF16)
        ones_f32 = persist.tile([1, 128], F32)
        ones_f = persist.tile([1, 64], F32)

        # main-loop pools pinned before setup so slots don't alias setup tiles
        MBUFS = int(os.environ.get("GAT_BUFS", "4"))
        mloop = ctx.enter_context(tc.tile_pool(name="mloop", bufs=MBUFS))
        for _b in range(MBUFS):
            _t = mloop.tile([128, QN], BF16, tag="mt")
            nc.vector.memset(_t[0:1, 0:2], 0.0)
            _t = mloop.tile([128, 2, QN], BF16, tag="mp23")
            nc.vector.memset(_t[0:1, 0, 0:2], 0.0)
            _t = mloop.tile([128, 2, QN], BF16, tag="ee")
            nc.vector.memset(_t[0:1, 0, 0:2], 0.0)
            _t = mloop.tile([128, 2, QN], BF16, tag="q23")
            nc.vector.memset(_t[0:1, 0, 0:2], 0.0)
            _t = mloop.tile([128, 2, QN], BF16, tag="pm01")
            nc.vector.memset(_t[0:1, 0, 0:2], 0.0)
            _t = mloop.tile([128, 2, QN], BF16, tag="pm23")
            nc.vector.memset(_t[0:1, 0, 0:2], 0.0)

        # ---------------- setup: DMAs + row broadcasts ----------------
        nc.vector.memset(ones_b, 1.0)
        nc.vector.memset(ones_f32, 1.0)
        nc.vector.memset(ones_f, 1.0)

        nc.scalar.dma_start(whv, whv_d[:, :].rearrange("p (k h d) -> p k h d", k=KB, h=H))
        nc.scalar.dma_start(b08, b08_d[:, :].rearrange("p (j k) -> p j k", j=2))
        nc.scalar.dma_start(wpt_sb, wpt_d[:, :].rearrange("(c p) w -> p c w", p=128))
        bp_ap = bp_d[:]
        nc.gpsimd.dma_start(bpb, bass.AP(tensor=bp_ap.tensor, offset=bp_ap.offset,
                                         ap=[[0, 128]] + list(bp_ap.ap)))

        WARMUP = int(os.environ.get("GAT_WARMUP", "24"))
        with tc.tile_pool(name="setup", bufs=1) as setup, \
             tc.tile_pool(name="spsum", bufs=4, space="PSUM") as spsum:
            a08row = setup.tile([1, 2, QN], F32)
            ea08row = setup.tile([1, 2, QN], BF16)
            nc.scalar.dma_start(a08row, a08_d[:, :].rearrange("(o j) q -> o j q", o=1))
            nc.scalar.dma_start(ea08row, ea08_d[:, :].rearrange("(o j) q -> o j q", o=1))
            # broadcast rows across 128 partitions via ones-matmuls
            for j in range(2):
                for qh in range(QH):
                    qsl = slice(qh * 512, (qh + 1) * 512)
                    pa = spsum.tile([128, 512], F32, tag="bc_a")
                    nc.tensor.matmul(pa, ones_f32, a08row[:, j, qsl])
                    nc.vector.tensor_copy(abc[:, j, qsl], pa)
                    pe = spsum.tile([128, 512], F32, tag="bc_e")
                    nc.tensor.matmul(pe, ones_b, ea08row[:, j, qsl])
                    nc.scalar.copy(eap23[:, j, qsl], pe)
            # PE warm-up: back-to-back dummy matmuls to flip HAM to 8/8
            # before the main accumulation begins
            for w in range(WARMUP):
                pw = spsum.tile([128, 512], F32, tag="bc_a")
                nc.tensor.matmul(pw[0:DH + 1, :], whv[:, 0, 0, :], eap23[:, 0, 0:512])

        # ---------------- main loop ----------------
        mpsum_cm = tc.tile_pool(name="mpsum", bufs=1, space="PSUM")
        mpsum = mpsum_cm.__enter__()
        acc = mpsum.tile([DH + 1, H, QH, 512], F32)

        # engine split: of the 128 mult pair-TT ops (2/block), TT_GPS go to
        # GPSIMD (Pool rejects max-TT), the rest to DVE.
        TT_GPS = int(os.environ.get("GAT_TT_GPS", "83"))  # per 128

        mi = 0

        def frac_hit(i, frac, tot):
            return (i * frac) // tot != ((i - 1) * frac) // tot

        def tt_engine():
            nonlocal mi
            mi += 1
            return nc.gpsimd if frac_hit(mi, TT_GPS, 128) else nc.vector

        for kb in range(KB):
            mt = mloop.tile([128, QN], BF16, tag="mt")
            nc.sync.dma_start(mt, adjt_d[kb * 128:(kb + 1) * 128, :])
            mt2 = bass.AP(tensor=mt.tensor, offset=mt.offset,
                          ap=[list(mt.ap[0]), [0, 2], list(mt.ap[1])])
            mp23 = mloop.tile([128, 2, QN], BF16, tag="mp23")
            nc.sync.dma_start(
                mp23, adjm_d[kb * 128:(kb + 1) * 128, :].rearrange(
                    "p (j q) -> p j q", j=2))

            # heads 0,1: ACT exp (per-partition bias), one flat 4x max, mask TT
            ee = mloop.tile([128, 2, QN], BF16, tag="ee")
            for j in range(2):
                nc.scalar.activation(ee[:, j, :], abc[:, j, :], AF.Exp,
                                     bias=b08[:, j, kb:kb + 1], scale=1.0)
            eeflat = bass.AP(tensor=ee.tensor, offset=ee.offset,
                             ap=[list(ee.ap[0]), [1, 2 * QN]])
            nc.vector.tensor_scalar(eeflat, eeflat, 1.0, None, op0=ALU.max)
            pm01 = mloop.tile([128, 2, QN], BF16, tag="pm01")
            tt_engine().tensor_tensor(pm01, ee, mt2, op=ALU.mult)

            # heads 2,3: vb08 pre-folded into adjm; pm = max(ea * madj, mt)
            q23 = mloop.tile([128, 2, QN], BF16, tag="q23")
            tt_engine().tensor_tensor(q23, eap23, mp23, op=ALU.mult)
            pm23 = mloop.tile([128, 2, QN], BF16, tag="pm23")
            nc.vector.tensor_tensor(pm23, q23, mt2, op=ALU.max)

            for hs in range(H):
                pm = pm01 if hs < 2 else pm23
                j = hs % 2
                for qh in range(QH):
                    nc.tensor.matmul(acc[:, hs, qh, :], whv[:, kb, hs, :],
                                     pm[:, j, qh * 512:(qh + 1) * 512],
                                     start=(kb == 0), stop=(kb == KB - 1))

        # ---------------- tail: normalize, elu, out-proj ----------------
        tailp = ctx.enter_context(tc.tile_pool(name="tailp", bufs=1))
        denr = tailp.tile([1, H, QN], F32)
        gfin = tailp.tile([128, 2, QN], F32)
        graw = tailp.tile([128, 2, QN], F32)
        ACT_RECIP = int(os.environ.get("GAT_ACT_RECIP", "1"))
        for hs in range(H):
            for qh in range(QH):
                qsl = slice(qh * 512, (qh + 1) * 512)
                if ACT_RECIP:
                    # 1/den = square(1/sqrt(den)) on ACT (den > 0), keeping
                    # the iterative-divide off the DVE critical path
                    nc.scalar.activation(denr[:, hs, qsl], acc[DH:DH + 1, hs, qh, :],
                                         AF.Abs_reciprocal_sqrt)
                else:
                    nc.vector.reciprocal(denr[:, hs, qsl], acc[DH:DH + 1, hs, qh, :])
            graw_dst = graw[(hs % 2) * 64:(hs % 2) * 64 + 64, hs // 2, :]
            graw_src = acc[0:DH, hs, :, :].rearrange("p a b -> p (a b)")
            if hs % 2 == 0:
                nc.scalar.copy(graw_dst, graw_src)
            else:
                nc.vector.tensor_copy(graw_dst, graw_src)
        if ACT_RECIP:
            nc.scalar.activation(denr[0:1, :, :], denr[0:1, :, :], AF.Square)
        mpsum_cm.__exit__(None, None, None)

        with tc.tile_pool(name="tpsum", bufs=2, space="PSUM") as tpsum:
            # normalize: broadcast 1/den across partitions via ones-matmul
            for j in range(2):
                for qh in range(QH):
                    qsl = slice(qh * 512, (qh + 1) * 512)
                    rps = tpsum.tile([128, 512], F32, tag="r_ps")
                    nc.tensor.matmul(rps[0:64, :], ones_f, denr[:, 2 * j, qsl])
                    nc.tensor.matmul(rps[64:128, :], ones_f, denr[:, 2 * j + 1, qsl])
                    nc.vector.tensor_mul(gfin[:, j, qsl], graw[:, j, qsl], rps)

            # elu(x) = relu(x) + exp(min(x, 0)) - 1
            for j in range(2):
                for qh in range(QH):
                    qsl = slice(qh * 512, (qh + 1) * 512)
                    t = tailp.tile([128, 512], F32, tag="elu_t")
                    nc.vector.tensor_scalar(t, gfin[:, j, qsl], 0.0, None,
                                            op0=ALU.min)
                    e = tailp.tile([128, 512], F32, tag="elu_e")
                    nc.scalar.activation(e, t, AF.Exp)
                    em1 = tailp.tile([128, 512], F32, tag="elu_em1")
                    nc.vector.tensor_scalar(em1, e, -1.0, None, op0=ALU.add)
                    nc.vector.scalar_tensor_tensor(gfin[:, j, qsl], gfin[:, j, qsl],
                                                   0.0, em1, op0=ALU.max, op1=ALU.add)

            for qc in range(QN // 128):
                qsl = slice(qc * 128, (qc + 1) * 128)
                po = tpsum.tile([128, IN_F], F32, tag="out_ps")
                nc.tensor.matmul(po, gfin[:, 0, qsl], wpt_sb[:, 0, :],
                                 start=True, stop=False)
                nc.tensor.matmul(po, gfin[:, 1, qsl], wpt_sb[:, 1, :],
                                 start=False, stop=True)
                fin = tailp.tile([128, IN_F], F32, tag="fin")
                nc.vector.scalar_tensor_tensor(fin, po, 0.0, bpb,
                                               op0=ALU.add, op1=ALU.add)
                nc.sync.dma_start(out[qsl, :], fin)

    nc.compile()
    return nc


_NC_CACHE = {}
LAST_RESULTS = None


def _get_nc():
    if "nc" not in _NC_CACHE:
        _NC_CACHE["nc"] = build_nc()
    return _NC_CACHE["nc"]


def kernel(h, adj, W, a1, a2, Wp, bp):
    from concourse.bass_utils import run_bass_kernel_spmd

    h = np.asarray(h, dtype=np.float32)
    adj = np.asarray(adj)
    W = np.asarray(W, dtype=np.float32)
    a1 = np.asarray(a1, dtype=np.float32)
    a2 = np.asarray(a2, dtype=np.float32)
    Wp = np.asarray(Wp, dtype=np.float32)
    bp = np.asarray(bp, dtype=np.float32)

    # ---- host precompute (O(N d^2): ~1% of kernel FLOPs) ----
    Wh = np.einsum("ni,hid->nhd", h, W).astype(np.float32)     # [N, H, DH]
    asc = np.einsum("nhd,hd->hn", Wh, a1)                      # [H, N]
    bsc = np.einsum("nhd,hd->hn", Wh, a2)                      # [H, N]
    vb02 = np.exp(0.2 * bsc)                                   # [H, N]
    vb08 = np.exp(0.8 * bsc)
    # value stationaries [128, KB, H, DH+1]: [Wh * vb02 | vb02]
    whv_f = np.concatenate(
        [Wh * vb02.T[:, :, None], vb02.T[:, :, None]], axis=2)  # [N, H, DH+1]
    whv_np = np.ascontiguousarray(
        whv_f.reshape(KB, 128, H, DH + 1).transpose(1, 0, 2, 3)
        .reshape(128, KB * H * (DH + 1)).astype(BF16_NP))
    b08_np = np.ascontiguousarray(
        (0.8 * bsc[0:2]).T.reshape(KB, 128, 2).transpose(1, 2, 0)
        .reshape(128, 2 * KB).astype(np.float32))
    wpt = np.ascontiguousarray(Wp.T)

    nc = _get_nc()
    in_maps = []
    for c in range(NCORES):
        qsl = slice(c * QN, (c + 1) * QN)
        adjt_f = adj[qsl, :].T.astype(np.float32)           # [N, QN]
        adjm = adjt_f[:, None, :] * vb08[2:4].T[:, :, None]  # [N, 2, QN]
        in_maps.append({
            "whv": whv_np,
            "adjt": adjt_f.astype(BF16_NP),
            "adjm": adjm.reshape(N, 2 * QN).astype(BF16_NP),
            "a08": np.ascontiguousarray(0.8 * asc[0:2, qsl]).astype(np.float32),
            "ea08": np.ascontiguousarray(np.exp(0.8 * asc[2:4, qsl])).astype(BF16_NP),
            "b08": b08_np,
            "wpt": wpt,
            "bp": bp,
        })

    res = run_bass_kernel_spmd(nc, in_maps, core_ids=list(range(NCORES)))
    global LAST_RESULTS
    LAST_RESULTS = res
    return np.concatenate([r["out"] for r in res.results], axis=0)


# revision 27
# speedup vs baseline: 1.4875x; 1.2770x over previous
"""Multi-head graph attention (GAT) Trainium2 kernel, v2.

Row-sharded across 8 NeuronCores: core i owns queries [i*1024, (i+1)*1024).

Math (per head h, with Wh = h @ W_h, a = Wh@a1, b = Wh@a2, s = a_i + b_j):
    e[i,j]  = leakyrelu(s, 0.2)
    attn    = softmax_j(where(adj>0, e, -9e15))
    out_h   = elu(attn @ Wh)
    out     = concat_h(out_h) @ Wp.T + bp

On-chip factorization (exact): exp(lrelu(s)) = exp(0.2s) * max(exp(0.8s), 1).
The per-query factor exp(0.2 a_i) cancels in softmax, so the unnormalized
weight used on-chip is
    w[j,i] = adjT[j,i] * vb02_j * max(exp(0.8 a_i + 0.8 b_j), 1)
with vb02_j = exp(0.2 b_j) folded into the value stationaries host-side.

Per key-block (128 keys x 1024 queries), per head the masked weights are
built one of two ways (to spread work across engines):
  ACT-form (heads 0,1):  e = ScalarE exp(abc + b08_j)  [per-partition bias],
                         pm = DVE stt: (e max 1) * mask
  z-form  (heads 2,3):   z = ea08b * mask   (TT mult, DVE/GPSIMD)
                         pm = (z * vb08_j) max mask    (stt, DVE/GPSIMD)
      since mask in {0,1}: max(z*vb08, mask) = mask * max(exp(.8s), 1).

All setup tensors (Wh, score rows, exp factors, scaled stationaries) are
precomputed on host; device setup is pure DMA + 8 small broadcast matmuls.
adj is host-transposed to bf16 so mask loads are plain contiguous DMAs.
"""

import os
from contextlib import ExitStack

import numpy as np
import ml_dtypes

import concourse.bacc as bacc
import concourse.bass as bass
import concourse.mybir as mybir
import concourse.tile as tile

F32 = mybir.dt.float32
BF16 = mybir.dt.bfloat16

ALU = mybir.AluOpType
AF = mybir.ActivationFunctionType

N = 8192          # nodes
IN_F = 256        # input features
H = 4             # heads
DH = 64           # head dim
NCORES = 8
QN = N // NCORES  # queries per core (1024)
KB = N // 128     # key blocks of 128 (64)
QH = 2            # 512-wide query halves

BF16_NP = ml_dtypes.bfloat16


def build_nc():
    nc = bacc.Bacc("TRN2", target_bir_lowering=False, debug=False)

    # host-precomputed tensors
    whv_d = nc.declare_dram_parameter("whv", [128, KB * H * (DH + 1)], BF16, False)
    adjt_d = nc.declare_dram_parameter("adjt", [N, QN], BF16, False)
    # heads 2,3: mask pre-scaled by vb08 = exp(0.8 b_j) host-side
    adjm_d = nc.declare_dram_parameter("adjm", [N, 2 * QN], BF16, False)
    a08_d = nc.declare_dram_parameter("a08", [2, QN], F32, False)      # heads 0,1: 0.8*a
    ea08_d = nc.declare_dram_parameter("ea08", [2, QN], BF16, False)   # heads 2,3: exp(0.8*a)
    b08_d = nc.declare_dram_parameter("b08", [128, 2 * KB], F32, False)    # heads 0,1
    wpt_d = nc.declare_dram_parameter("wpt", [IN_F, IN_F], F32, False)  # Wp.T
    bp_d = nc.declare_dram_parameter("bp", [IN_F], F32, False)
    out = nc.declare_dram_parameter("out", [QN, IN_F], F32, True)

    with ExitStack() as ctx:
        tc = ctx.enter_context(tile.TileContext(nc))

        persist = ctx.enter_context(tc.tile_pool(name="persist", bufs=1))
        whv = persist.tile([128, KB, H, DH + 1], BF16)
        abc = persist.tile([128, 2, QN], F32)      # broadcast 0.8*a rows, heads 0,1
        eap23 = persist.tile([128, 2, QN], BF16)   # broadcast exp(0.8a), heads 2,3
        b08 = persist.tile([128, 2, KB], F32)
        wpt_sb = persist.tile([128, 2, IN_F], F32)
        bpb = persist.tile([128, IN_F], F32)
        ones_b = persist.tile([1, 128], BF16)
        ones_f32 = persist.tile([1, 128], F32)
        ones_f = persist.tile([1, 64], F32)

        # main-loop pools pinned before setup so slots don't alias setup tiles
        MBUFS = int(os.environ.get("GAT_BUFS", "4"))
        mloop = ctx.enter_context(tc.tile_pool(name="mloop", bufs=MBUFS))
        for _b in range(MBUFS):
            _t = mloop.tile([128, QN], BF16, tag="mt")
            nc.vector.memset(_t[0:1, 0:2], 0.0)
            _t = mloop.tile([128, 2, QN], BF16, tag="mp23")
            nc.vector.memset(_t[0:1, 0, 0:2], 0.0)
            _t = mloop.tile([128, 2, QN], BF16, tag="ee")
            nc.vector.memset(_t[0:1, 0, 0:2], 0.0)
            _t = mloop.tile([128, 2, QN], BF16, tag="q23")
            nc.vector.memset(_t[0:1, 0, 0:2], 0.0)
            _t = mloop.tile([128, 2, QN], BF16, tag="pm01")
            nc.vector.memset(_t[0:1, 0, 0:2], 0.0)
            _t = mloop.tile([128, 2, QN], BF16, tag="pm23")
            nc.vector.memset(_t[0:1, 0, 0:2], 0.0)

        # ---------------- setup: DMAs + row broadcasts ----------------
        nc.vector.memset(ones_b, 1.0)
        nc.vector.memset(ones_f32, 1.0)
        nc.vector.memset(ones_f, 1.0)

        nc.scalar.dma_start(whv, whv_d[:, :].rearrange("p (k h d) -> p k h d", k=KB, h=H))
        nc.scalar.dma_start(b08, b08_d[:, :].rearrange("p (j k) -> p j k", j=2))
        nc.scalar.dma_start(wpt_sb, wpt_d[:, :].rearrange("(c p) w -> p c w", p=128))
        bp_ap = bp_d[:]
        nc.gpsimd.dma_start(bpb, bass.AP(tensor=bp_ap.tensor, offset=bp_ap.offset,
                                         ap=[[0, 128]] + list(bp_ap.ap)))

        WARMUP = int(os.environ.get("GAT_WARMUP", "24"))
        with tc.tile_pool(name="setup", bufs=1) as setup, \
             tc.tile_pool(name="spsum", bufs=4, space="PSUM") as spsum:
            a08row = setup.tile([1, 2, QN], F32)
            ea08row = setup.tile([1, 2, QN], BF16)
            nc.scalar.dma_start(a08row, a08_d[:, :].rearrange("(o j) q -> o j q", o=1))
            nc.scalar.dma_start(ea08row, ea08_d[:, :].rearrange("(o j) q -> o j q", o=1))
            # broadcast rows across 128 partitions via ones-matmuls
            for j in range(2):
                for qh in range(QH):
                    qsl = slice(qh * 512, (qh + 1) * 512)
                    pa = spsum.tile([128, 512], F32, tag="bc_a")
                    nc.tensor.matmul(pa, ones_f32, a08row[:, j, qsl])
                    nc.vector.tensor_copy(abc[:, j, qsl], pa)
                    pe = spsum.tile([128, 512], F32, tag="bc_e")
                    nc.tensor.matmul(pe, ones_b, ea08row[:, j, qsl])
                    nc.scalar.copy(eap23[:, j, qsl], pe)
            # PE warm-up: back-to-back dummy matmuls to flip HAM to 8/8
            # before the main accumulation begins
            for w in range(WARMUP):
                pw = spsum.tile([128, 512], F32, tag="bc_a")
                nc.tensor.matmul(pw[0:DH + 1, :], whv[:, 0, 0, :], eap23[:, 0, 0:512])

        # ---------------- main loop ----------------
        mpsum_cm = tc.tile_pool(name="mpsum", bufs=1, space="PSUM")
        mpsum = mpsum_cm.__enter__()
        acc = mpsum.tile([DH + 1, H, QH, 512], F32)

        # engine split: of the 128 mult pair-TT ops (2/block), TT_GPS go to
        # GPSIMD (Pool rejects max-TT), the rest to DVE.
        TT_GPS = int(os.environ.get("GAT_TT_GPS", "83"))  # per 128

        mi = 0

        def frac_hit(i, frac, tot):
            return (i * frac) // tot != ((i - 1) * frac) // tot

        def tt_engine():
            nonlocal mi
            mi += 1
            return nc.gpsimd if frac_hit(mi, TT_GPS, 128) else nc.vector

        # software pipeline: the pm23 max (DVE) and all matmuls for block kb
        # are emitted DELAY iterations later, so the strict-FIFO DVE and PE
        # queues never head-of-line-block on a slow (GPSIMD) producer.
        DELAY = int(os.environ.get("GAT_DELAY", "2"))
        pend = []

        def finish_block(item):
            kb0, pm01_0, q23_0, mt2_0 = item
            pm23 = mloop.tile([128, 2, QN], BF16, tag="pm23")
            nc.vector.tensor_tensor(pm23, q23_0, mt2_0, op=ALU.max)
            for hs in range(H):
                pm = pm01_0 if hs < 2 else pm23
                j = hs % 2
                for qh in range(QH):
                    nc.tensor.matmul(acc[:, hs, qh, :], whv[:, kb0, hs, :],
                                     pm[:, j, qh * 512:(qh + 1) * 512],
                                     start=(kb0 == 0), stop=(kb0 == KB - 1))

        for kb in range(KB):
            mt = mloop.tile([128, QN], BF16, tag="mt")
            nc.sync.dma_start(mt, adjt_d[kb * 128:(kb + 1) * 128, :])
            mt2 = bass.AP(tensor=mt.tensor, offset=mt.offset,
                          ap=[list(mt.ap[0]), [0, 2], list(mt.ap[1])])
            mp23 = mloop.tile([128, 2, QN], BF16, tag="mp23")
            nc.sync.dma_start(
                mp23, adjm_d[kb * 128:(kb + 1) * 128, :].rearrange(
                    "p (j q) -> p j q", j=2))

            # heads 0,1: ACT exp (per-partition bias), one flat 4x max, mask TT
            ee = mloop.tile([128, 2, QN], BF16, tag="ee")
            for j in range(2):
                nc.scalar.activation(ee[:, j, :], abc[:, j, :], AF.Exp,
                                     bias=b08[:, j, kb:kb + 1], scale=1.0)
            eeflat = bass.AP(tensor=ee.tensor, offset=ee.offset,
                             ap=[list(ee.ap[0]), [1, 2 * QN]])
            nc.vector.tensor_scalar(eeflat, eeflat, 1.0, None, op0=ALU.max)
            pm01 = mloop.tile([128, 2, QN], BF16, tag="pm01")
            tt_engine().tensor_tensor(pm01, ee, mt2, op=ALU.mult)

            # heads 2,3: vb08 pre-folded into adjm; pm = max(ea * madj, mt)
            q23 = mloop.tile([128, 2, QN], BF16, tag="q23")
            tt_engine().tensor_tensor(q23, eap23, mp23, op=ALU.mult)

            pend.append((kb, pm01, q23, mt2))
            if len(pend) > DELAY:
                finish_block(pend.pop(0))

        for item in pend:
            finish_block(item)

        # ---------------- tail: normalize, elu, out-proj ----------------
        tailp = ctx.enter_context(tc.tile_pool(name="tailp", bufs=1))
        denr = tailp.tile([1, H, QN], F32)
        gfin = tailp.tile([128, 2, QN], F32)
        graw = tailp.tile([128, 2, QN], F32)
        ACT_RECIP = int(os.environ.get("GAT_ACT_RECIP", "1"))
        for hs in range(H):
            for qh in range(QH):
                qsl = slice(qh * 512, (qh + 1) * 512)
                if ACT_RECIP:
                    # 1/den = square(1/sqrt(den)) on ACT (den > 0), keeping
                    # the iterative-divide off the DVE critical path
                    nc.scalar.activation(denr[:, hs, qsl], acc[DH:DH + 1, hs, qh, :],
                                         AF.Abs_reciprocal_sqrt)
                else:
                    nc.vector.reciprocal(denr[:, hs, qsl], acc[DH:DH + 1, hs, qh, :])
            graw_dst = graw[(hs % 2) * 64:(hs % 2) * 64 + 64, hs // 2, :]
            graw_src = acc[0:DH, hs, :, :].rearrange("p a b -> p (a b)")
            if hs % 2 == 0:
                nc.scalar.copy(graw_dst, graw_src)
            else:
                nc.vector.tensor_copy(graw_dst, graw_src)
        if ACT_RECIP:
            nc.scalar.activation(denr[0:1, :, :], denr[0:1, :, :], AF.Square)
        mpsum_cm.__exit__(None, None, None)

        with tc.tile_pool(name="tpsum", bufs=2, space="PSUM") as tpsum:
            # normalize: broadcast 1/den across partitions via ones-matmul
            for j in range(2):
                for qh in range(QH):
                    qsl = slice(qh * 512, (qh + 1) * 512)
                    rps = tpsum.tile([128, 512], F32, tag="r_ps")
                    nc.tensor.matmul(rps[0:64, :], ones_f, denr[:, 2 * j, qsl])
                    nc.tensor.matmul(rps[64:128, :], ones_f, denr[:, 2 * j + 1, qsl])
                    nc.vector.tensor_mul(gfin[:, j, qsl], graw[:, j, qsl], rps)

            # elu(x) = relu(x) + exp(min(x, 0)) - 1
            for j in range(2):
                for qh in range(QH):
                    qsl = slice(qh * 512, (qh + 1) * 512)
                    t = tailp.tile([128, 512], F32, tag="elu_t")
                    nc.vector.tensor_scalar(t, gfin[:, j, qsl], 0.0, None,
                                            op0=ALU.min)
                    e = tailp.tile([128, 512], F32, tag="elu_e")
                    nc.scalar.activation(e, t, AF.Exp)
                    em1 = tailp.tile([128, 512], F32, tag="elu_em1")
                    nc.vector.tensor_scalar(em1, e, -1.0, None, op0=ALU.add)
                    nc.vector.scalar_tensor_tensor(gfin[:, j, qsl], gfin[:, j, qsl],
                                                   0.0, em1, op0=ALU.max, op1=ALU.add)

            for qc in range(QN // 128):
                qsl = slice(qc * 128, (qc + 1) * 128)
                po = tpsum.tile([128, IN_F], F32, tag="out_ps")
                nc.tensor.matmul(po, gfin[:, 0, qsl], wpt_sb[:, 0, :],
                                 start=True, stop=False)
                nc.tensor.matmul(po, gfin[:, 1, qsl], wpt_sb[:, 1, :],
                                 start=False, stop=True)
                fin = tailp.tile([128, IN_F], F32, tag="fin")
                nc.vector.scalar_tensor_tensor(fin, po, 0.0, bpb,
                                               op0=ALU.add, op1=ALU.add)
                nc.sync.dma_start(out[qsl, :], fin)

    nc.compile()
    return nc


_NC_CACHE = {}
LAST_RESULTS = None


def _get_nc():
    if "nc" not in _NC_CACHE:
        _NC_CACHE["nc"] = build_nc()
    return _NC_CACHE["nc"]


def kernel(h, adj, W, a1, a2, Wp, bp):
    from concourse.bass_utils import run_bass_kernel_spmd

    h = np.asarray(h, dtype=np.float32)
    adj = np.asarray(adj)
    W = np.asarray(W, dtype=np.float32)
    a1 = np.asarray(a1, dtype=np.float32)
    a2 = np.asarray(a2, dtype=np.float32)
    Wp = np.asarray(Wp, dtype=np.float32)
    bp = np.asarray(bp, dtype=np.float32)

    # ---- host precompute (O(N d^2): ~1% of kernel FLOPs) ----
    Wh = np.einsum("ni,hid->nhd", h, W).astype(np.float32)     # [N, H, DH]
    asc = np.einsum("nhd,hd->hn", Wh, a1)                      # [H, N]
    bsc = np.einsum("nhd,hd->hn", Wh, a2)                      # [H, N]
    vb02 = np.exp(0.2 * bsc)                                   # [H, N]
    vb08 = np.exp(0.8 * bsc)
    # value stationaries [128, KB, H, DH+1]: [Wh * vb02 | vb02]
    whv_f = np.concatenate(
        [Wh * vb02.T[:, :, None], vb02.T[:, :, None]], axis=2)  # [N, H, DH+1]
    whv_np = np.ascontiguousarray(
        whv_f.reshape(KB, 128, H, DH + 1).transpose(1, 0, 2, 3)
        .reshape(128, KB * H * (DH + 1)).astype(BF16_NP))
    b08_np = np.ascontiguousarray(
        (0.8 * bsc[0:2]).T.reshape(KB, 128, 2).transpose(1, 2, 0)
        .reshape(128, 2 * KB).astype(np.float32))
    wpt = np.ascontiguousarray(Wp.T)

    nc = _get_nc()
    in_maps = []
    for c in range(NCORES):
        qsl = slice(c * QN, (c + 1) * QN)
        adjt_f = adj[qsl, :].T.astype(np.float32)           # [N, QN]
        adjm = adjt_f[:, None, :] * vb08[2:4].T[:, :, None]  # [N, 2, QN]
        in_maps.append({
            "whv": whv_np,
            "adjt": adjt_f.astype(BF16_NP),
            "adjm": adjm.reshape(N, 2 * QN).astype(BF16_NP),
            "a08": np.ascontiguousarray(0.8 * asc[0:2, qsl]).astype(np.float32),
            "ea08": np.ascontiguousarray(np.exp(0.8 * asc[2:4, qsl])).astype(BF16_NP),
            "b08": b08_np,
            "wpt": wpt,
            "bp": bp,
        })

    res = run_bass_kernel_spmd(nc, in_maps, core_ids=list(range(NCORES)))
    global LAST_RESULTS
    LAST_RESULTS = res
    return np.concatenate([r["out"] for r in res.results], axis=0)


# revision 30
# speedup vs baseline: 1.5294x; 1.0282x over previous
"""Multi-head graph attention (GAT) Trainium2 kernel, v2.

Row-sharded across 8 NeuronCores: core i owns queries [i*1024, (i+1)*1024).

Math (per head h, with Wh = h @ W_h, a = Wh@a1, b = Wh@a2, s = a_i + b_j):
    e[i,j]  = leakyrelu(s, 0.2)
    attn    = softmax_j(where(adj>0, e, -9e15))
    out_h   = elu(attn @ Wh)
    out     = concat_h(out_h) @ Wp.T + bp

On-chip factorization (exact): exp(lrelu(s)) = exp(0.2s) * max(exp(0.8s), 1).
The per-query factor exp(0.2 a_i) cancels in softmax, so the unnormalized
weight used on-chip is
    w[j,i] = adjT[j,i] * vb02_j * max(exp(0.8 a_i + 0.8 b_j), 1)
with vb02_j = exp(0.2 b_j) folded into the value stationaries host-side.

Per key-block (128 keys x 1024 queries), per head the masked weights are
built one of two ways (to spread work across engines):
  ACT-form (heads 0,1):  e = ScalarE exp(abc + b08_j)  [per-partition bias],
                         pm = DVE stt: (e max 1) * mask
  z-form  (heads 2,3):   z = ea08b * mask   (TT mult, DVE/GPSIMD)
                         pm = (z * vb08_j) max mask    (stt, DVE/GPSIMD)
      since mask in {0,1}: max(z*vb08, mask) = mask * max(exp(.8s), 1).

All setup tensors (Wh, score rows, exp factors, scaled stationaries) are
precomputed on host; device setup is pure DMA + 8 small broadcast matmuls.
adj is host-transposed to bf16 so mask loads are plain contiguous DMAs.
"""

import os
from contextlib import ExitStack

import numpy as np
import ml_dtypes

import concourse.bacc as bacc
import concourse.bass as bass
import concourse.mybir as mybir
import concourse.tile as tile

F32 = mybir.dt.float32
BF16 = mybir.dt.bfloat16

ALU = mybir.AluOpType
AF = mybir.ActivationFunctionType

N = 8192          # nodes
IN_F = 256        # input features
H = 4             # heads
DH = 64           # head dim
NCORES = 8
QN = N // NCORES  # queries per core (1024)
KB = N // 128     # key blocks of 128 (64)
QH = 2            # 512-wide query halves

BF16_NP = ml_dtypes.bfloat16


def build_nc():
    nc = bacc.Bacc("TRN2", target_bir_lowering=False, debug=False)

    # host-precomputed tensors
    whv_d = nc.declare_dram_parameter("whv", [128, KB * H * (DH + 1)], BF16, False)
    adjt_d = nc.declare_dram_parameter("adjt", [N, QN], BF16, False)
    # heads 2,3: mask pre-scaled by vb08 = exp(0.8 b_j) host-side
    adjm_d = nc.declare_dram_parameter("adjm", [N, 2 * QN], BF16, False)
    a08_d = nc.declare_dram_parameter("a08", [2, QN], F32, False)      # heads 0,1: 0.8*a
    ea08_d = nc.declare_dram_parameter("ea08", [2, QN], BF16, False)   # heads 2,3: exp(0.8*a)
    b08_d = nc.declare_dram_parameter("b08", [128, 2 * KB], F32, False)    # heads 0,1
    wpt_d = nc.declare_dram_parameter("wpt", [IN_F, IN_F], F32, False)  # Wp.T
    bp_d = nc.declare_dram_parameter("bp", [IN_F], F32, False)
    out = nc.declare_dram_parameter("out", [QN, IN_F], F32, True)

    with ExitStack() as ctx:
        tc = ctx.enter_context(tile.TileContext(nc))

        persist = ctx.enter_context(tc.tile_pool(name="persist", bufs=1))
        whv = persist.tile([128, KB, H, DH + 1], BF16)
        abc = persist.tile([128, 2, QN], F32)      # broadcast 0.8*a rows, heads 0,1
        eap23 = persist.tile([128, 2, QN], BF16)   # broadcast exp(0.8a), heads 2,3
        b08 = persist.tile([128, 2, KB], F32)
        wpt_sb = persist.tile([128, 2, IN_F], F32)
        bpb = persist.tile([128, IN_F], F32)
        ones_b = persist.tile([1, 128], BF16)
        ones_f32 = persist.tile([1, 128], F32)
        ones_f = persist.tile([1, 64], F32)

        # main-loop pools pinned before setup so slots don't alias setup tiles
        MBUFS = int(os.environ.get("GAT_BUFS", "4"))
        mloop = ctx.enter_context(tc.tile_pool(name="mloop", bufs=MBUFS))
        for _b in range(MBUFS):
            _t = mloop.tile([128, QN], BF16, tag="mt")
            nc.vector.memset(_t[0:1, 0:2], 0.0)
            _t = mloop.tile([128, 2, QN], BF16, tag="mp23")
            nc.vector.memset(_t[0:1, 0, 0:2], 0.0)
            _t = mloop.tile([128, 2, QN], BF16, tag="ee")
            nc.vector.memset(_t[0:1, 0, 0:2], 0.0)
            _t = mloop.tile([128, 2, QN], BF16, tag="q23")
            nc.vector.memset(_t[0:1, 0, 0:2], 0.0)
            _t = mloop.tile([128, 2, QN], BF16, tag="pm01")
            nc.vector.memset(_t[0:1, 0, 0:2], 0.0)
            _t = mloop.tile([128, 2, QN], BF16, tag="pm23")
            nc.vector.memset(_t[0:1, 0, 0:2], 0.0)

        # ---------------- setup: DMAs + row broadcasts ----------------
        nc.vector.memset(ones_b, 1.0)
        nc.vector.memset(ones_f32, 1.0)
        nc.vector.memset(ones_f, 1.0)

        nc.scalar.dma_start(b08, b08_d[:, :].rearrange("p (j k) -> p j k", j=2))
        nc.scalar.dma_start(wpt_sb, wpt_d[:, :].rearrange("(c p) w -> p c w", p=128))
        bp_ap = bp_d[:]
        nc.gpsimd.dma_start(bpb, bass.AP(tensor=bp_ap.tensor, offset=bp_ap.offset,
                                         ap=[[0, 128]] + list(bp_ap.ap)))
        # whv streamed in key-block chunks so the first main matmuls are not
        # gated on the full 4.25 MB stationary load
        whv_r = whv_d[:, :].rearrange("p (k h d) -> p k h d", k=KB, h=H)
        for wc in range(8):
            ks = slice(wc * (KB // 8), (wc + 1) * (KB // 8))
            nc.scalar.dma_start(whv[:, ks, :, :], whv_r[:, ks, :, :])

        WARMUP = int(os.environ.get("GAT_WARMUP", "16"))
        with tc.tile_pool(name="setup", bufs=1) as setup, \
             tc.tile_pool(name="spsum", bufs=4, space="PSUM") as spsum:
            a08row = setup.tile([1, 2, QN], F32)
            ea08row = setup.tile([1, 2, QN], BF16)
            nc.sync.dma_start(a08row, a08_d[:, :].rearrange("(o j) q -> o j q", o=1))
            nc.sync.dma_start(ea08row, ea08_d[:, :].rearrange("(o j) q -> o j q", o=1))
            # PE warm-up: back-to-back dummy matmuls (inputs depend only on
            # two small early DMAs) to flip HAM to 8/8 before the real MMs
            for w in range(WARMUP):
                pw = spsum.tile([128, 512], F32, tag="bc_a")
                nc.tensor.matmul(pw[:, 0:256], wpt_sb[:, 0, 0:128], bpb)
            # broadcast rows across 128 partitions via ones-matmuls
            for j in range(2):
                for qh in range(QH):
                    qsl = slice(qh * 512, (qh + 1) * 512)
                    pa = spsum.tile([128, 512], F32, tag="bc_a")
                    nc.tensor.matmul(pa, ones_f32, a08row[:, j, qsl])
                    nc.vector.tensor_copy(abc[:, j, qsl], pa)
                    pe = spsum.tile([128, 512], F32, tag="bc_e")
                    nc.tensor.matmul(pe, ones_b, ea08row[:, j, qsl])
                    nc.scalar.copy(eap23[:, j, qsl], pe)

        # ---------------- main loop ----------------
        mpsum_cm = tc.tile_pool(name="mpsum", bufs=1, space="PSUM")
        mpsum = mpsum_cm.__enter__()
        acc = mpsum.tile([DH + 1, H, QH, 512], F32)

        # engine split: of the 128 mult pair-TT ops (2/block), TT_GPS go to
        # GPSIMD (Pool rejects max-TT), the rest to DVE.
        TT_GPS = int(os.environ.get("GAT_TT_GPS", "0"))  # per 128

        mi = 0

        def frac_hit(i, frac, tot):
            return (i * frac) // tot != ((i - 1) * frac) // tot

        def tt_engine():
            nonlocal mi
            mi += 1
            return nc.gpsimd if frac_hit(mi, TT_GPS, 128) else nc.vector

        # software pipeline: the pm23 max (DVE) and all matmuls for block kb
        # are emitted DELAY iterations later, so the strict-FIFO DVE and PE
        # queues never head-of-line-block on a slow (GPSIMD) producer.
        DELAY = int(os.environ.get("GAT_DELAY", "2"))
        pend = []

        def finish_block(item):
            kb0, pm01_0, q23_0, mt2_0 = item
            pm23 = mloop.tile([128, 2, QN], BF16, tag="pm23")
            nc.vector.tensor_tensor(pm23, q23_0, mt2_0, op=ALU.max)
            for hs in range(H):
                pm = pm01_0 if hs < 2 else pm23
                j = hs % 2
                for qh in range(QH):
                    nc.tensor.matmul(acc[:, hs, qh, :], whv[:, kb0, hs, :],
                                     pm[:, j, qh * 512:(qh + 1) * 512],
                                     start=(kb0 == 0), stop=(kb0 == KB - 1))

        for kb in range(KB):
            mt = mloop.tile([128, QN], BF16, tag="mt")
            nc.sync.dma_start(mt, adjt_d[kb * 128:(kb + 1) * 128, :])
            mt2 = bass.AP(tensor=mt.tensor, offset=mt.offset,
                          ap=[list(mt.ap[0]), [0, 2], list(mt.ap[1])])
            mp23 = mloop.tile([128, 2, QN], BF16, tag="mp23")
            nc.sync.dma_start(
                mp23, adjm_d[kb * 128:(kb + 1) * 128, :].rearrange(
                    "p (j q) -> p j q", j=2))

            # heads 0,1: ACT exp (per-partition bias), one flat 4x max, mask TT
            ee = mloop.tile([128, 2, QN], BF16, tag="ee")
            for j in range(2):
                nc.scalar.activation(ee[:, j, :], abc[:, j, :], AF.Exp,
                                     bias=b08[:, j, kb:kb + 1], scale=1.0)
            eeflat = bass.AP(tensor=ee.tensor, offset=ee.offset,
                             ap=[list(ee.ap[0]), [1, 2 * QN]])
            nc.vector.tensor_scalar(eeflat, eeflat, 1.0, None, op0=ALU.max)
            pm01 = mloop.tile([128, 2, QN], BF16, tag="pm01")
            tt_engine().tensor_tensor(pm01, ee, mt2, op=ALU.mult)

            # heads 2,3: vb08 pre-folded into adjm; pm = max(ea * madj, mt)
            q23 = mloop.tile([128, 2, QN], BF16, tag="q23")
            tt_engine().tensor_tensor(q23, eap23, mp23, op=ALU.mult)

            pend.append((kb, pm01, q23, mt2))
            if len(pend) > DELAY:
                finish_block(pend.pop(0))

        for item in pend:
            finish_block(item)

        # ---------------- tail: normalize, elu, out-proj ----------------
        tailp = ctx.enter_context(tc.tile_pool(name="tailp", bufs=1))
        denr = tailp.tile([1, H, QN], F32)
        gfin = tailp.tile([128, 2, QN], F32)
        graw = tailp.tile([128, 2, QN], F32)
        ACT_RECIP = int(os.environ.get("GAT_ACT_RECIP", "1"))
        for hs in range(H):
            for qh in range(QH):
                qsl = slice(qh * 512, (qh + 1) * 512)
                if ACT_RECIP:
                    # 1/den = square(1/sqrt(den)) on ACT (den > 0), keeping
                    # the iterative-divide off the DVE critical path
                    nc.scalar.activation(denr[:, hs, qsl], acc[DH:DH + 1, hs, qh, :],
                                         AF.Abs_reciprocal_sqrt)
                else:
                    nc.vector.reciprocal(denr[:, hs, qsl], acc[DH:DH + 1, hs, qh, :])
            graw_dst = graw[(hs % 2) * 64:(hs % 2) * 64 + 64, hs // 2, :]
            graw_src = acc[0:DH, hs, :, :].rearrange("p a b -> p (a b)")
            if hs % 2 == 0:
                nc.scalar.copy(graw_dst, graw_src)
            else:
                nc.vector.tensor_copy(graw_dst, graw_src)
        if ACT_RECIP:
            nc.scalar.activation(denr[0:1, :, :], denr[0:1, :, :], AF.Square)
        mpsum_cm.__exit__(None, None, None)

        with tc.tile_pool(name="tpsum", bufs=2, space="PSUM") as tpsum:
            # normalize: broadcast 1/den across partitions via ones-matmul
            for j in range(2):
                for qh in range(QH):
                    qsl = slice(qh * 512, (qh + 1) * 512)
                    rps = tpsum.tile([128, 512], F32, tag="r_ps")
                    nc.tensor.matmul(rps[0:64, :], ones_f, denr[:, 2 * j, qsl])
                    nc.tensor.matmul(rps[64:128, :], ones_f, denr[:, 2 * j + 1, qsl])
                    nc.vector.tensor_mul(gfin[:, j, qsl], graw[:, j, qsl], rps)

            # elu(x) = relu(x) + exp(min(x, 0)) - 1
            for j in range(2):
                for qh in range(QH):
                    qsl = slice(qh * 512, (qh + 1) * 512)
                    t = tailp.tile([128, 512], F32, tag="elu_t")
                    nc.vector.tensor_scalar(t, gfin[:, j, qsl], 0.0, None,
                                            op0=ALU.min)
                    e = tailp.tile([128, 512], F32, tag="elu_e")
                    nc.scalar.activation(e, t, AF.Exp)
                    em1 = tailp.tile([128, 512], F32, tag="elu_em1")
                    nc.vector.tensor_scalar(em1, e, -1.0, None, op0=ALU.add)
                    nc.vector.scalar_tensor_tensor(gfin[:, j, qsl], gfin[:, j, qsl],
                                                   0.0, em1, op0=ALU.max, op1=ALU.add)

            for qc in range(QN // 128):
                qsl = slice(qc * 128, (qc + 1) * 128)
                po = tpsum.tile([128, IN_F], F32, tag="out_ps")
                nc.tensor.matmul(po, gfin[:, 0, qsl], wpt_sb[:, 0, :],
                                 start=True, stop=False)
                nc.tensor.matmul(po, gfin[:, 1, qsl], wpt_sb[:, 1, :],
                                 start=False, stop=True)
                fin = tailp.tile([128, IN_F], F32, tag="fin")
                nc.vector.scalar_tensor_tensor(fin, po, 0.0, bpb,
                                               op0=ALU.add, op1=ALU.add)
                nc.sync.dma_start(out[qsl, :], fin)

    nc.compile()
    return nc


_NC_CACHE = {}
LAST_RESULTS = None


def _get_nc():
    if "nc" not in _NC_CACHE:
        _NC_CACHE["nc"] = build_nc()
    return _NC_CACHE["nc"]


def kernel(h, adj, W, a1, a2, Wp, bp):
    from concourse.bass_utils import run_bass_kernel_spmd

    h = np.asarray(h, dtype=np.float32)
    adj = np.asarray(adj)
    W = np.asarray(W, dtype=np.float32)
    a1 = np.asarray(a1, dtype=np.float32)
    a2 = np.asarray(a2, dtype=np.float32)
    Wp = np.asarray(Wp, dtype=np.float32)
    bp = np.asarray(bp, dtype=np.float32)

    # ---- host precompute (O(N d^2): ~1% of kernel FLOPs) ----
    Wh = np.einsum("ni,hid->nhd", h, W).astype(np.float32)     # [N, H, DH]
    asc = np.einsum("nhd,hd->hn", Wh, a1)                      # [H, N]
    bsc = np.einsum("nhd,hd->hn", Wh, a2)                      # [H, N]
    vb02 = np.exp(0.2 * bsc)                                   # [H, N]
    vb08 = np.exp(0.8 * bsc)
    # value stationaries [128, KB, H, DH+1]: [Wh * vb02 | vb02]
    whv_f = np.concatenate(
        [Wh * vb02.T[:, :, None], vb02.T[:, :, None]], axis=2)  # [N, H, DH+1]
    whv_np = np.ascontiguousarray(
        whv_f.reshape(KB, 128, H, DH + 1).transpose(1, 0, 2, 3)
        .reshape(128, KB * H * (DH + 1)).astype(BF16_NP))
    b08_np = np.ascontiguousarray(
        (0.8 * bsc[0:2]).T.reshape(KB, 128, 2).transpose(1, 2, 0)
        .reshape(128, 2 * KB).astype(np.float32))
    wpt = np.ascontiguousarray(Wp.T)

    nc = _get_nc()
    in_maps = []
    for c in range(NCORES):
        qsl = slice(c * QN, (c + 1) * QN)
        adjt_f = adj[qsl, :].T.astype(np.float32)           # [N, QN]
        adjm = adjt_f[:, None, :] * vb08[2:4].T[:, :, None]  # [N, 2, QN]
        in_maps.append({
            "whv": whv_np,
            "adjt": adjt_f.astype(BF16_NP),
            "adjm": adjm.reshape(N, 2 * QN).astype(BF16_NP),
            "a08": np.ascontiguousarray(0.8 * asc[0:2, qsl]).astype(np.float32),
            "ea08": np.ascontiguousarray(np.exp(0.8 * asc[2:4, qsl])).astype(BF16_NP),
            "b08": b08_np,
            "wpt": wpt,
            "bp": bp,
        })

    res = run_bass_kernel_spmd(nc, in_maps, core_ids=list(range(NCORES)))
    global LAST_RESULTS
    LAST_RESULTS = res
    return np.concatenate([r["out"] for r in res.results], axis=0)


# revision 31
# speedup vs baseline: 1.5376x; 1.0054x over previous
"""Multi-head graph attention (GAT) Trainium2 kernel, v2.

Row-sharded across 8 NeuronCores: core i owns queries [i*1024, (i+1)*1024).

Math (per head h, with Wh = h @ W_h, a = Wh@a1, b = Wh@a2, s = a_i + b_j):
    e[i,j]  = leakyrelu(s, 0.2)
    attn    = softmax_j(where(adj>0, e, -9e15))
    out_h   = elu(attn @ Wh)
    out     = concat_h(out_h) @ Wp.T + bp

On-chip factorization (exact): exp(lrelu(s)) = exp(0.2s) * max(exp(0.8s), 1).
The per-query factor exp(0.2 a_i) cancels in softmax, so the unnormalized
weight used on-chip is
    w[j,i] = adjT[j,i] * vb02_j * max(exp(0.8 a_i + 0.8 b_j), 1)
with vb02_j = exp(0.2 b_j) folded into the value stationaries host-side.

Per key-block (128 keys x 1024 queries), per head the masked weights are
built one of two ways (to spread work across engines):
  ACT-form (heads 0,1):  e = ScalarE exp(abc + b08_j)  [per-partition bias],
                         pm = DVE stt: (e max 1) * mask
  z-form  (heads 2,3):   z = ea08b * mask   (TT mult, DVE/GPSIMD)
                         pm = (z * vb08_j) max mask    (stt, DVE/GPSIMD)
      since mask in {0,1}: max(z*vb08, mask) = mask * max(exp(.8s), 1).

All setup tensors (Wh, score rows, exp factors, scaled stationaries) are
precomputed on host; device setup is pure DMA + 8 small broadcast matmuls.
adj is host-transposed to bf16 so mask loads are plain contiguous DMAs.
"""

import os
from contextlib import ExitStack

import numpy as np
import ml_dtypes

import concourse.bacc as bacc
import concourse.bass as bass
import concourse.mybir as mybir
import concourse.tile as tile

F32 = mybir.dt.float32
BF16 = mybir.dt.bfloat16

ALU = mybir.AluOpType
AF = mybir.ActivationFunctionType

N = 8192          # nodes
IN_F = 256        # input features
H = 4             # heads
DH = 64           # head dim
NCORES = 8
QN = N // NCORES  # queries per core (1024)
KB = N // 128     # key blocks of 128 (64)
QH = 2            # 512-wide query halves

BF16_NP = ml_dtypes.bfloat16


def build_nc():
    nc = bacc.Bacc("TRN2", target_bir_lowering=False, debug=False)

    # host-precomputed tensors
    whv_d = nc.declare_dram_parameter("whv", [128, KB * H * (DH + 1)], BF16, False)
    adjt_d = nc.declare_dram_parameter("adjt", [N, QN], BF16, False)
    # heads 2,3: mask pre-scaled by vb08 = exp(0.8 b_j) host-side
    adjm_d = nc.declare_dram_parameter("adjm", [N, 2 * QN], BF16, False)
    a08_d = nc.declare_dram_parameter("a08", [2, QN], F32, False)      # heads 0,1: 0.8*a
    ea08_d = nc.declare_dram_parameter("ea08", [2, QN], BF16, False)   # heads 2,3: exp(0.8*a)
    b08_d = nc.declare_dram_parameter("b08", [128, 2 * KB], F32, False)    # heads 0,1
    wpt_d = nc.declare_dram_parameter("wpt", [IN_F, IN_F], F32, False)  # Wp.T
    bp_d = nc.declare_dram_parameter("bp", [IN_F], F32, False)
    out = nc.declare_dram_parameter("out", [QN, IN_F], F32, True)

    with ExitStack() as ctx:
        tc = ctx.enter_context(tile.TileContext(nc))

        persist = ctx.enter_context(tc.tile_pool(name="persist", bufs=1))
        whv = persist.tile([128, KB, H, DH + 1], BF16)
        abc = persist.tile([128, 2, QN], F32)      # broadcast 0.8*a rows, heads 0,1
        eap23 = persist.tile([128, 2, QN], BF16)   # broadcast exp(0.8a), heads 2,3
        b08 = persist.tile([128, 2, KB], F32)
        wpt_sb = persist.tile([128, 2, IN_F], F32)
        bpb = persist.tile([128, IN_F], F32)
        ones_b = persist.tile([1, 128], BF16)
        ones_f32 = persist.tile([1, 128], F32)
        ones_f = persist.tile([1, 64], F32)

        # main-loop pools pinned before setup so slots don't alias setup tiles
        MBUFS = int(os.environ.get("GAT_BUFS", "4"))
        mloop = ctx.enter_context(tc.tile_pool(name="mloop", bufs=MBUFS))
        for _b in range(MBUFS):
            _t = mloop.tile([128, QN], BF16, tag="mt")
            nc.vector.memset(_t[0:1, 0:2], 0.0)
            _t = mloop.tile([128, 2, QN], BF16, tag="mp23")
            nc.vector.memset(_t[0:1, 0, 0:2], 0.0)
            _t = mloop.tile([128, 2, QN], BF16, tag="ee")
            nc.vector.memset(_t[0:1, 0, 0:2], 0.0)
            _t = mloop.tile([128, 2, QN], BF16, tag="q23")
            nc.vector.memset(_t[0:1, 0, 0:2], 0.0)
            _t = mloop.tile([128, 2, QN], BF16, tag="pm01")
            nc.vector.memset(_t[0:1, 0, 0:2], 0.0)
            _t = mloop.tile([128, 2, QN], BF16, tag="pm23")
            nc.vector.memset(_t[0:1, 0, 0:2], 0.0)

        # ---------------- setup: DMAs + row broadcasts ----------------
        nc.vector.memset(ones_b, 1.0)
        nc.vector.memset(ones_f32, 1.0)
        nc.vector.memset(ones_f, 1.0)

        nc.scalar.dma_start(b08, b08_d[:, :].rearrange("p (j k) -> p j k", j=2))
        nc.scalar.dma_start(wpt_sb, wpt_d[:, :].rearrange("(c p) w -> p c w", p=128))
        bp_ap = bp_d[:]
        nc.gpsimd.dma_start(bpb, bass.AP(tensor=bp_ap.tensor, offset=bp_ap.offset,
                                         ap=[[0, 128]] + list(bp_ap.ap)))
        # whv streamed in key-block chunks so the first main matmuls are not
        # gated on the full 4.25 MB stationary load
        whv_r = whv_d[:, :].rearrange("p (k h d) -> p k h d", k=KB, h=H)
        for wc in range(8):
            ks = slice(wc * (KB // 8), (wc + 1) * (KB // 8))
            nc.scalar.dma_start(whv[:, ks, :, :], whv_r[:, ks, :, :])

        WARMUP = int(os.environ.get("GAT_WARMUP", "16"))
        with tc.tile_pool(name="setup", bufs=1) as setup, \
             tc.tile_pool(name="spsum", bufs=4, space="PSUM") as spsum:
            a08row = setup.tile([1, 2, QN], F32)
            ea08row = setup.tile([1, 2, QN], BF16)
            nc.sync.dma_start(a08row, a08_d[:, :].rearrange("(o j) q -> o j q", o=1))
            nc.sync.dma_start(ea08row, ea08_d[:, :].rearrange("(o j) q -> o j q", o=1))
            # PE warm-up: back-to-back dummy matmuls (inputs depend only on
            # two small early DMAs) to flip HAM to 8/8 before the real MMs
            for w in range(WARMUP):
                pw = spsum.tile([128, 512], F32, tag="bc_a")
                nc.tensor.matmul(pw[:, 0:256], wpt_sb[:, 0, 0:128], wpt_sb[:, 1, :])
            # broadcast rows across 128 partitions via ones-matmuls
            for j in range(2):
                for qh in range(QH):
                    qsl = slice(qh * 512, (qh + 1) * 512)
                    pa = spsum.tile([128, 512], F32, tag="bc_a")
                    nc.tensor.matmul(pa, ones_f32, a08row[:, j, qsl])
                    nc.vector.tensor_copy(abc[:, j, qsl], pa)
                    pe = spsum.tile([128, 512], F32, tag="bc_e")
                    nc.tensor.matmul(pe, ones_b, ea08row[:, j, qsl])
                    nc.scalar.copy(eap23[:, j, qsl], pe)

        # ---------------- main loop ----------------
        mpsum_cm = tc.tile_pool(name="mpsum", bufs=1, space="PSUM")
        mpsum = mpsum_cm.__enter__()
        acc = mpsum.tile([DH + 1, H, QH, 512], F32)

        # engine split: of the 128 mult pair-TT ops (2/block), TT_GPS go to
        # GPSIMD (Pool rejects max-TT), the rest to DVE.
        TT_GPS = int(os.environ.get("GAT_TT_GPS", "0"))  # per 128

        mi = 0

        def frac_hit(i, frac, tot):
            return (i * frac) // tot != ((i - 1) * frac) // tot

        def tt_engine():
            nonlocal mi
            mi += 1
            return nc.gpsimd if frac_hit(mi, TT_GPS, 128) else nc.vector

        # software pipeline: the pm23 max (DVE) and all matmuls for block kb
        # are emitted DELAY iterations later, so the strict-FIFO DVE and PE
        # queues never head-of-line-block on a slow (GPSIMD) producer.
        DELAY = int(os.environ.get("GAT_DELAY", "2"))
        pend = []

        def finish_block(item):
            kb0, pm01_0, q23_0, mt2_0 = item
            pm23 = mloop.tile([128, 2, QN], BF16, tag="pm23")
            nc.vector.tensor_tensor(pm23, q23_0, mt2_0, op=ALU.max)
            for hs in range(H):
                pm = pm01_0 if hs < 2 else pm23
                j = hs % 2
                for qh in range(QH):
                    nc.tensor.matmul(acc[:, hs, qh, :], whv[:, kb0, hs, :],
                                     pm[:, j, qh * 512:(qh + 1) * 512],
                                     start=(kb0 == 0), stop=(kb0 == KB - 1))

        for kb in range(KB):
            mt = mloop.tile([128, QN], BF16, tag="mt")
            nc.sync.dma_start(mt, adjt_d[kb * 128:(kb + 1) * 128, :])
            mt2 = bass.AP(tensor=mt.tensor, offset=mt.offset,
                          ap=[list(mt.ap[0]), [0, 2], list(mt.ap[1])])
            mp23 = mloop.tile([128, 2, QN], BF16, tag="mp23")
            nc.sync.dma_start(
                mp23, adjm_d[kb * 128:(kb + 1) * 128, :].rearrange(
                    "p (j q) -> p j q", j=2))

            # heads 0,1: ACT exp (per-partition bias), one flat 4x max, mask TT
            ee = mloop.tile([128, 2, QN], BF16, tag="ee")
            for j in range(2):
                nc.scalar.activation(ee[:, j, :], abc[:, j, :], AF.Exp,
                                     bias=b08[:, j, kb:kb + 1], scale=1.0)
            eeflat = bass.AP(tensor=ee.tensor, offset=ee.offset,
                             ap=[list(ee.ap[0]), [1, 2 * QN]])
            nc.vector.tensor_scalar(eeflat, eeflat, 1.0, None, op0=ALU.max)
            pm01 = mloop.tile([128, 2, QN], BF16, tag="pm01")
            tt_engine().tensor_tensor(pm01, ee, mt2, op=ALU.mult)

            # heads 2,3: vb08 pre-folded into adjm; pm = max(ea * madj, mt)
            q23 = mloop.tile([128, 2, QN], BF16, tag="q23")
            tt_engine().tensor_tensor(q23, eap23, mp23, op=ALU.mult)

            pend.append((kb, pm01, q23, mt2))
            if len(pend) > DELAY:
                finish_block(pend.pop(0))

        for item in pend:
            finish_block(item)

        # ---------------- tail: normalize, elu, out-proj ----------------
        tailp = ctx.enter_context(tc.tile_pool(name="tailp", bufs=1))
        denr = tailp.tile([1, H, QN], F32)
        gfin = tailp.tile([128, 2, QN], F32)
        graw = tailp.tile([128, 2, QN], F32)
        ACT_RECIP = int(os.environ.get("GAT_ACT_RECIP", "1"))
        for hs in range(H):
            for qh in range(QH):
                qsl = slice(qh * 512, (qh + 1) * 512)
                if ACT_RECIP:
                    # 1/den = square(1/sqrt(den)) on ACT (den > 0), keeping
                    # the iterative-divide off the DVE critical path
                    nc.scalar.activation(denr[:, hs, qsl], acc[DH:DH + 1, hs, qh, :],
                                         AF.Abs_reciprocal_sqrt)
                    nc.scalar.activation(denr[:, hs, qsl], denr[:, hs, qsl],
                                         AF.Square)
                else:
                    nc.vector.reciprocal(denr[:, hs, qsl], acc[DH:DH + 1, hs, qh, :])
            graw_dst = graw[(hs % 2) * 64:(hs % 2) * 64 + 64, hs // 2, :]
            graw_src = acc[0:DH, hs, :, :].rearrange("p a b -> p (a b)")
            if hs % 2 == 0:
                nc.scalar.copy(graw_dst, graw_src)
            else:
                nc.vector.tensor_copy(graw_dst, graw_src)
        mpsum_cm.__exit__(None, None, None)

        with tc.tile_pool(name="tpsum", bufs=2, space="PSUM") as tpsum:
            # normalize: broadcast 1/den across partitions via ones-matmul
            for j in range(2):
                for qh in range(QH):
                    qsl = slice(qh * 512, (qh + 1) * 512)
                    rps = tpsum.tile([128, 512], F32, tag="r_ps")
                    nc.tensor.matmul(rps[0:64, :], ones_f, denr[:, 2 * j, qsl])
                    nc.tensor.matmul(rps[64:128, :], ones_f, denr[:, 2 * j + 1, qsl])
                    nc.vector.tensor_mul(gfin[:, j, qsl], graw[:, j, qsl], rps)

            # elu(x) = relu(x) + exp(min(x, 0)) - 1
            for qh in range(QH):
                for j in range(2):
                    qsl = slice(qh * 512, (qh + 1) * 512)
                    t = tailp.tile([128, 512], F32, tag="elu_t")
                    nc.vector.tensor_scalar(t, gfin[:, j, qsl], 0.0, None,
                                            op0=ALU.min)
                    e = tailp.tile([128, 512], F32, tag="elu_e")
                    nc.scalar.activation(e, t, AF.Exp)
                    em1 = tailp.tile([128, 512], F32, tag="elu_em1")
                    nc.vector.tensor_scalar(em1, e, -1.0, None, op0=ALU.add)
                    nc.vector.scalar_tensor_tensor(gfin[:, j, qsl], gfin[:, j, qsl],
                                                   0.0, em1, op0=ALU.max, op1=ALU.add)

            for qc in range(QN // 128):
                qsl = slice(qc * 128, (qc + 1) * 128)
                po = tpsum.tile([128, IN_F], F32, tag="out_ps")
                nc.tensor.matmul(po, gfin[:, 0, qsl], wpt_sb[:, 0, :],
                                 start=True, stop=False)
                nc.tensor.matmul(po, gfin[:, 1, qsl], wpt_sb[:, 1, :],
                                 start=False, stop=True)
                fin = tailp.tile([128, IN_F], F32, tag="fin")
                nc.vector.scalar_tensor_tensor(fin, po, 0.0, bpb,
                                               op0=ALU.add, op1=ALU.add)
                nc.sync.dma_start(out[qsl, :], fin)

    nc.compile()
    return nc


_NC_CACHE = {}
LAST_RESULTS = None


def _get_nc():
    if "nc" not in _NC_CACHE:
        _NC_CACHE["nc"] = build_nc()
    return _NC_CACHE["nc"]


def kernel(h, adj, W, a1, a2, Wp, bp):
    from concourse.bass_utils import run_bass_kernel_spmd

    h = np.asarray(h, dtype=np.float32)
    adj = np.asarray(adj)
    W = np.asarray(W, dtype=np.float32)
    a1 = np.asarray(a1, dtype=np.float32)
    a2 = np.asarray(a2, dtype=np.float32)
    Wp = np.asarray(Wp, dtype=np.float32)
    bp = np.asarray(bp, dtype=np.float32)

    # ---- host precompute (O(N d^2): ~1% of kernel FLOPs) ----
    Wh = np.einsum("ni,hid->nhd", h, W).astype(np.float32)     # [N, H, DH]
    asc = np.einsum("nhd,hd->hn", Wh, a1)                      # [H, N]
    bsc = np.einsum("nhd,hd->hn", Wh, a2)                      # [H, N]
    vb02 = np.exp(0.2 * bsc)                                   # [H, N]
    vb08 = np.exp(0.8 * bsc)
    # value stationaries [128, KB, H, DH+1]: [Wh * vb02 | vb02]
    whv_f = np.concatenate(
        [Wh * vb02.T[:, :, None], vb02.T[:, :, None]], axis=2)  # [N, H, DH+1]
    whv_np = np.ascontiguousarray(
        whv_f.reshape(KB, 128, H, DH + 1).transpose(1, 0, 2, 3)
        .reshape(128, KB * H * (DH + 1)).astype(BF16_NP))
    b08_np = np.ascontiguousarray(
        (0.8 * bsc[0:2]).T.reshape(KB, 128, 2).transpose(1, 2, 0)
        .reshape(128, 2 * KB).astype(np.float32))
    wpt = np.ascontiguousarray(Wp.T)

    nc = _get_nc()
    in_maps = []
    for c in range(NCORES):
        qsl = slice(c * QN, (c + 1) * QN)
        adjt_f = adj[qsl, :].T.astype(np.float32)           # [N, QN]
        adjm = adjt_f[:, None, :] * vb08[2:4].T[:, :, None]  # [N, 2, QN]
        in_maps.append({
            "whv": whv_np,
            "adjt": adjt_f.astype(BF16_NP),
            "adjm": adjm.reshape(N, 2 * QN).astype(BF16_NP),
            "a08": np.ascontiguousarray(0.8 * asc[0:2, qsl]).astype(np.float32),
            "ea08": np.ascontiguousarray(np.exp(0.8 * asc[2:4, qsl])).astype(BF16_NP),
            "b08": b08_np,
            "wpt": wpt,
            "bp": bp,
        })

    res = run_bass_kernel_spmd(nc, in_maps, core_ids=list(range(NCORES)))
    global LAST_RESULTS
    LAST_RESULTS = res
    return np.concatenate([r["out"] for r in res.results], axis=0)


# revision 33
# speedup vs baseline: 1.6091x; 1.0465x over previous
"""Multi-head graph attention (GAT) Trainium2 kernel, v2.

Row-sharded across 8 NeuronCores: core i owns queries [i*1024, (i+1)*1024).

Math (per head h, with Wh = h @ W_h, a = Wh@a1, b = Wh@a2, s = a_i + b_j):
    e[i,j]  = leakyrelu(s, 0.2)
    attn    = softmax_j(where(adj>0, e, -9e15))
    out_h   = elu(attn @ Wh)
    out     = concat_h(out_h) @ Wp.T + bp

On-chip factorization (exact): exp(lrelu(s)) = exp(0.2s) * max(exp(0.8s), 1).
The per-query factor exp(0.2 a_i) cancels in softmax, so the unnormalized
weight used on-chip is
    w[j,i] = adjT[j,i] * vb02_j * max(exp(0.8 a_i + 0.8 b_j), 1)
with vb02_j = exp(0.2 b_j) folded into the value stationaries host-side.

Per key-block (128 keys x 1024 queries), per head the masked weights are
built one of two ways (to spread work across engines):
  ACT-form (heads 0,1):  e = ScalarE exp(abc + b08_j)  [per-partition bias],
                         pm = DVE stt: (e max 1) * mask
  z-form  (heads 2,3):   z = ea08b * mask   (TT mult, DVE/GPSIMD)
                         pm = (z * vb08_j) max mask    (stt, DVE/GPSIMD)
      since mask in {0,1}: max(z*vb08, mask) = mask * max(exp(.8s), 1).

All setup tensors (Wh, score rows, exp factors, scaled stationaries) are
precomputed on host; device setup is pure DMA + 8 small broadcast matmuls.
adj is host-transposed to bf16 so mask loads are plain contiguous DMAs.
"""

import os
from contextlib import ExitStack

import numpy as np
import ml_dtypes

import concourse.bacc as bacc
import concourse.bass as bass
import concourse.mybir as mybir
import concourse.tile as tile

F32 = mybir.dt.float32
BF16 = mybir.dt.bfloat16

ALU = mybir.AluOpType
AF = mybir.ActivationFunctionType

N = 8192          # nodes
IN_F = 256        # input features
H = 4             # heads
DH = 64           # head dim
NCORES = 8
QN = N // NCORES  # queries per core (1024)
KB = N // 128     # key blocks of 128 (64)
QH = 2            # 512-wide query halves

BF16_NP = ml_dtypes.bfloat16


def build_nc():
    nc = bacc.Bacc("TRN2", target_bir_lowering=False, debug=False)

    # host-precomputed tensors
    whv_d = nc.declare_dram_parameter("whv", [128, KB * H * (DH + 1)], BF16, False)
    adjt_d = nc.declare_dram_parameter("adjt", [N, QN], BF16, False)
    # heads 2,3: mask pre-scaled by vb08 = exp(0.8 b_j) host-side
    adjm_d = nc.declare_dram_parameter("adjm", [N, 2 * QN], BF16, False)
    a08_d = nc.declare_dram_parameter("a08", [4, QN], F32, False)      # all heads: 0.8*a
    ea08_d = nc.declare_dram_parameter("ea08", [2, QN], BF16, False)   # heads 2,3: exp(0.8*a)
    b08_d = nc.declare_dram_parameter("b08", [128, 4 * KB], F32, False)    # all heads
    wpt_d = nc.declare_dram_parameter("wpt", [IN_F, IN_F], F32, False)  # Wp.T
    bp_d = nc.declare_dram_parameter("bp", [IN_F], F32, False)
    out = nc.declare_dram_parameter("out", [QN, IN_F], F32, True)

    with ExitStack() as ctx:
        tc = ctx.enter_context(tile.TileContext(nc))

        persist = ctx.enter_context(tc.tile_pool(name="persist", bufs=1))
        whv = persist.tile([128, KB, H, DH + 1], BF16)
        abc = persist.tile([128, 4, QN], F32)      # broadcast 0.8*a rows, all heads
        eap23 = persist.tile([128, 2, QN], BF16)   # broadcast exp(0.8a), heads 2,3
        b08 = persist.tile([128, 4, KB], F32)
        wpt_sb = persist.tile([128, 2, IN_F], F32)
        bpb = persist.tile([128, IN_F], F32)
        ones_b = persist.tile([1, 128], BF16)
        ones_f32 = persist.tile([1, 128], F32)
        ones_f = persist.tile([1, 64], F32)

        # main-loop pools pinned before setup so slots don't alias setup tiles
        MBUFS = int(os.environ.get("GAT_BUFS", "4"))
        mloop = ctx.enter_context(tc.tile_pool(name="mloop", bufs=MBUFS))
        for _b in range(MBUFS):
            _t = mloop.tile([128, QN], BF16, tag="mt")
            nc.vector.memset(_t[0:1, 0:2], 0.0)
            _t = mloop.tile([128, 2, QN], BF16, tag="mp23")
            nc.vector.memset(_t[0:1, 0, 0:2], 0.0)
            _t = mloop.tile([128, 4, QN], BF16, tag="ee4")
            nc.vector.memset(_t[0:1, 0, 0:2], 0.0)
            _t = mloop.tile([128, 4, QN], BF16, tag="pm4")
            nc.vector.memset(_t[0:1, 0, 0:2], 0.0)

        # ---------------- setup: DMAs + row broadcasts ----------------
        nc.vector.memset(ones_b, 1.0)
        nc.vector.memset(ones_f32, 1.0)
        nc.vector.memset(ones_f, 1.0)

        nc.scalar.dma_start(b08, b08_d[:, :].rearrange("p (j k) -> p j k", j=4))
        nc.scalar.dma_start(wpt_sb, wpt_d[:, :].rearrange("(c p) w -> p c w", p=128))
        bp_ap = bp_d[:]
        nc.gpsimd.dma_start(bpb, bass.AP(tensor=bp_ap.tensor, offset=bp_ap.offset,
                                         ap=[[0, 128]] + list(bp_ap.ap)))
        # whv streamed in key-block chunks so the first main matmuls are not
        # gated on the full 4.25 MB stationary load
        whv_r = whv_d[:, :].rearrange("p (k h d) -> p k h d", k=KB, h=H)
        for wc in range(8):
            ks = slice(wc * (KB // 8), (wc + 1) * (KB // 8))
            nc.scalar.dma_start(whv[:, ks, :, :], whv_r[:, ks, :, :])

        WARMUP = int(os.environ.get("GAT_WARMUP", "16"))
        with tc.tile_pool(name="setup", bufs=1) as setup, \
             tc.tile_pool(name="spsum", bufs=4, space="PSUM") as spsum:
            a08row = setup.tile([1, 4, QN], F32)
            ea08row = setup.tile([1, 2, QN], BF16)
            nc.sync.dma_start(a08row, a08_d[:, :].rearrange("(o j) q -> o j q", o=1))
            nc.sync.dma_start(ea08row, ea08_d[:, :].rearrange("(o j) q -> o j q", o=1))
            # PE warm-up: back-to-back dummy matmuls (inputs depend only on
            # two small early DMAs) to flip HAM to 8/8 before the real MMs
            for w in range(WARMUP):
                pw = spsum.tile([128, 512], F32, tag="bc_a")
                nc.tensor.matmul(pw[:, 0:256], wpt_sb[:, 0, 0:128], wpt_sb[:, 1, :])
            # broadcast rows across 128 partitions via ones-matmuls
            for j in range(4):
                for qh in range(QH):
                    qsl = slice(qh * 512, (qh + 1) * 512)
                    pa = spsum.tile([128, 512], F32, tag="bc_a")
                    nc.tensor.matmul(pa, ones_f32, a08row[:, j, qsl])
                    nc.vector.tensor_copy(abc[:, j, qsl], pa)
            for j in range(2):
                for qh in range(QH):
                    qsl = slice(qh * 512, (qh + 1) * 512)
                    pe = spsum.tile([128, 512], F32, tag="bc_e")
                    nc.tensor.matmul(pe, ones_b, ea08row[:, j, qsl])
                    nc.scalar.copy(eap23[:, j, qsl], pe)

        # ---------------- main loop ----------------
        mpsum_cm = tc.tile_pool(name="mpsum", bufs=1, space="PSUM")
        mpsum = mpsum_cm.__enter__()
        acc = mpsum.tile([DH + 1, H, QH, 512], F32)

        # engine split: of the 128 mult pair-TT ops (2/block), TT_GPS go to
        # GPSIMD (Pool rejects max-TT), the rest to DVE.
        TT_GPS = int(os.environ.get("GAT_TT_GPS", "0"))  # per 128

        mi = 0

        def frac_hit(i, frac, tot):
            return (i * frac) // tot != ((i - 1) * frac) // tot

        def tt_engine():
            nonlocal mi
            mi += 1
            return nc.gpsimd if frac_hit(mi, TT_GPS, 128) else nc.vector

        # of 64 blocks, ACT4 use the all-ACT form (4 exps + one flat max +
        # one 4-plane mask mult); the rest use the split form (2 exps +
        # madj-masks for heads 2,3).
        ACT4 = int(os.environ.get("GAT_ACT4", "35"))  # per 64

        # software pipeline: the pm23 max (DVE) and all matmuls for block kb
        # are emitted DELAY iterations later, so the strict-FIFO DVE and PE
        # queues never head-of-line-block on a slow producer.
        DELAY = int(os.environ.get("GAT_DELAY", "2"))
        pend = []

        def finish_block(item):
            kb0, pm4_0, ee4_0, mt2_0 = item
            if ee4_0 is not None:
                # split-form deferred stage: mask-max for heads 2,3
                nc.vector.tensor_tensor(pm4_0[:, 2:4, :], ee4_0[:, 2:4, :],
                                        mt2_0, op=ALU.max)
            for hs in range(H):
                for qh in range(QH):
                    nc.tensor.matmul(acc[:, hs, qh, :], whv[:, kb0, hs, :],
                                     pm4_0[:, hs, qh * 512:(qh + 1) * 512],
                                     start=(kb0 == 0), stop=(kb0 == KB - 1))

        for kb in range(KB):
            act4 = frac_hit(kb + 1, ACT4, 64)
            mt = mloop.tile([128, QN], BF16, tag="mt")
            nc.sync.dma_start(mt, adjt_d[kb * 128:(kb + 1) * 128, :])
            mt2 = bass.AP(tensor=mt.tensor, offset=mt.offset,
                          ap=[list(mt.ap[0]), [0, 2], list(mt.ap[1])])
            mt4 = bass.AP(tensor=mt.tensor, offset=mt.offset,
                          ap=[list(mt.ap[0]), [0, 4], list(mt.ap[1])])
            ee4 = mloop.tile([128, 4, QN], BF16, tag="ee4")
            pm4 = mloop.tile([128, 4, QN], BF16, tag="pm4")

            if act4:
                for j in range(4):
                    nc.scalar.activation(ee4[:, j, :], abc[:, j, :], AF.Exp,
                                         bias=b08[:, j, kb:kb + 1], scale=1.0)
                eeflat = bass.AP(tensor=ee4.tensor, offset=ee4.offset,
                                 ap=[list(ee4.ap[0]), [1, 4 * QN]])
                nc.vector.tensor_scalar(eeflat, eeflat, 1.0, None, op0=ALU.max)
                nc.vector.tensor_tensor(pm4, ee4, mt4, op=ALU.mult)
                pend.append((kb, pm4, None, None))
            else:
                mp23 = mloop.tile([128, 2, QN], BF16, tag="mp23")
                nc.sync.dma_start(
                    mp23, adjm_d[kb * 128:(kb + 1) * 128, :].rearrange(
                        "p (j q) -> p j q", j=2))
                for j in range(2):
                    nc.scalar.activation(ee4[:, j, :], abc[:, j, :], AF.Exp,
                                         bias=b08[:, j, kb:kb + 1], scale=1.0)
                eeflat = bass.AP(tensor=ee4.tensor, offset=ee4.offset,
                                 ap=[list(ee4.ap[0]), [1, 2 * QN]])
                nc.vector.tensor_scalar(eeflat, eeflat, 1.0, None, op0=ALU.max)
                nc.vector.tensor_tensor(pm4[:, 0:2, :], ee4[:, 0:2, :], mt2,
                                        op=ALU.mult)
                # heads 2,3: q = ea * madj into ee4 slots 2:4
                nc.vector.tensor_tensor(ee4[:, 2:4, :], eap23, mp23,
                                        op=ALU.mult)
                pend.append((kb, pm4, ee4, mt2))

            if len(pend) > DELAY:
                finish_block(pend.pop(0))

        for item in pend:
            finish_block(item)

        # ---------------- tail: normalize, elu, out-proj ----------------
        tailp = ctx.enter_context(tc.tile_pool(name="tailp", bufs=1))
        denr = tailp.tile([1, H, QN], F32)
        gfin = tailp.tile([128, 2, QN], F32)
        graw = tailp.tile([128, 2, QN], F32)
        ACT_RECIP = int(os.environ.get("GAT_ACT_RECIP", "1"))
        for hs in range(H):
            for qh in range(QH):
                qsl = slice(qh * 512, (qh + 1) * 512)
                if ACT_RECIP:
                    # 1/den = square(1/sqrt(den)) on ACT (den > 0), keeping
                    # the iterative-divide off the DVE critical path
                    nc.scalar.activation(denr[:, hs, qsl], acc[DH:DH + 1, hs, qh, :],
                                         AF.Abs_reciprocal_sqrt)
                    nc.scalar.activation(denr[:, hs, qsl], denr[:, hs, qsl],
                                         AF.Square)
                else:
                    nc.vector.reciprocal(denr[:, hs, qsl], acc[DH:DH + 1, hs, qh, :])
            graw_dst = graw[(hs % 2) * 64:(hs % 2) * 64 + 64, hs // 2, :]
            graw_src = acc[0:DH, hs, :, :].rearrange("p a b -> p (a b)")
            if hs % 2 == 0:
                nc.scalar.copy(graw_dst, graw_src)
            else:
                nc.vector.tensor_copy(graw_dst, graw_src)
        mpsum_cm.__exit__(None, None, None)

        with tc.tile_pool(name="tpsum", bufs=2, space="PSUM") as tpsum:
            # normalize: broadcast 1/den across partitions via ones-matmul
            for j in range(2):
                for qh in range(QH):
                    qsl = slice(qh * 512, (qh + 1) * 512)
                    rps = tpsum.tile([128, 512], F32, tag="r_ps")
                    nc.tensor.matmul(rps[0:64, :], ones_f, denr[:, 2 * j, qsl])
                    nc.tensor.matmul(rps[64:128, :], ones_f, denr[:, 2 * j + 1, qsl])
                    nc.vector.tensor_mul(gfin[:, j, qsl], graw[:, j, qsl], rps)

            # elu(x) = relu(x) + exp(min(x, 0)) - 1
            for qh in range(QH):
                for j in range(2):
                    qsl = slice(qh * 512, (qh + 1) * 512)
                    t = tailp.tile([128, 512], F32, tag="elu_t")
                    nc.vector.tensor_scalar(t, gfin[:, j, qsl], 0.0, None,
                                            op0=ALU.min)
                    e = tailp.tile([128, 512], F32, tag="elu_e")
                    nc.scalar.activation(e, t, AF.Exp)
                    em1 = tailp.tile([128, 512], F32, tag="elu_em1")
                    nc.vector.tensor_scalar(em1, e, -1.0, None, op0=ALU.add)
                    nc.vector.scalar_tensor_tensor(gfin[:, j, qsl], gfin[:, j, qsl],
                                                   0.0, em1, op0=ALU.max, op1=ALU.add)

            for qc in range(QN // 128):
                qsl = slice(qc * 128, (qc + 1) * 128)
                po = tpsum.tile([128, IN_F], F32, tag="out_ps")
                nc.tensor.matmul(po, gfin[:, 0, qsl], wpt_sb[:, 0, :],
                                 start=True, stop=False)
                nc.tensor.matmul(po, gfin[:, 1, qsl], wpt_sb[:, 1, :],
                                 start=False, stop=True)
                fin = tailp.tile([128, IN_F], F32, tag="fin")
                nc.vector.scalar_tensor_tensor(fin, po, 0.0, bpb,
                                               op0=ALU.add, op1=ALU.add)
                nc.sync.dma_start(out[qsl, :], fin)

    nc.compile()
    return nc


_NC_CACHE = {}
LAST_RESULTS = None


def _get_nc():
    if "nc" not in _NC_CACHE:
        _NC_CACHE["nc"] = build_nc()
    return _NC_CACHE["nc"]


def kernel(h, adj, W, a1, a2, Wp, bp):
    from concourse.bass_utils import run_bass_kernel_spmd

    h = np.asarray(h, dtype=np.float32)
    adj = np.asarray(adj)
    W = np.asarray(W, dtype=np.float32)
    a1 = np.asarray(a1, dtype=np.float32)
    a2 = np.asarray(a2, dtype=np.float32)
    Wp = np.asarray(Wp, dtype=np.float32)
    bp = np.asarray(bp, dtype=np.float32)

    # ---- host precompute (O(N d^2): ~1% of kernel FLOPs) ----
    Wh = np.einsum("ni,hid->nhd", h, W).astype(np.float32)     # [N, H, DH]
    asc = np.einsum("nhd,hd->hn", Wh, a1)                      # [H, N]
    bsc = np.einsum("nhd,hd->hn", Wh, a2)                      # [H, N]
    vb02 = np.exp(0.2 * bsc)                                   # [H, N]
    vb08 = np.exp(0.8 * bsc)
    # value stationaries [128, KB, H, DH+1]: [Wh * vb02 | vb02]
    whv_f = np.concatenate(
        [Wh * vb02.T[:, :, None], vb02.T[:, :, None]], axis=2)  # [N, H, DH+1]
    whv_np = np.ascontiguousarray(
        whv_f.reshape(KB, 128, H, DH + 1).transpose(1, 0, 2, 3)
        .reshape(128, KB * H * (DH + 1)).astype(BF16_NP))
    b08_np = np.ascontiguousarray(
        (0.8 * bsc).T.reshape(KB, 128, H).transpose(1, 2, 0)
        .reshape(128, H * KB).astype(np.float32))
    wpt = np.ascontiguousarray(Wp.T)

    nc = _get_nc()
    in_maps = []
    for c in range(NCORES):
        qsl = slice(c * QN, (c + 1) * QN)
        adjt_f = adj[qsl, :].T.astype(np.float32)           # [N, QN]
        adjm = adjt_f[:, None, :] * vb08[2:4].T[:, :, None]  # [N, 2, QN]
        in_maps.append({
            "whv": whv_np,
            "adjt": adjt_f.astype(BF16_NP),
            "adjm": adjm.reshape(N, 2 * QN).astype(BF16_NP),
            "a08": np.ascontiguousarray(0.8 * asc[:, qsl]).astype(np.float32),
            "ea08": np.ascontiguousarray(np.exp(0.8 * asc[2:4, qsl])).astype(BF16_NP),
            "b08": b08_np,
            "wpt": wpt,
            "bp": bp,
        })

    res = run_bass_kernel_spmd(nc, in_maps, core_ids=list(range(NCORES)))
    global LAST_RESULTS
    LAST_RESULTS = res
    return np.concatenate([r["out"] for r in res.results], axis=0)


# revision 34
# speedup vs baseline: 1.6460x; 1.0229x over previous
"""Multi-head graph attention (GAT) Trainium2 kernel, v2.

Row-sharded across 8 NeuronCores: core i owns queries [i*1024, (i+1)*1024).

Math (per head h, with Wh = h @ W_h, a = Wh@a1, b = Wh@a2, s = a_i + b_j):
    e[i,j]  = leakyrelu(s, 0.2)
    attn    = softmax_j(where(adj>0, e, -9e15))
    out_h   = elu(attn @ Wh)
    out     = concat_h(out_h) @ Wp.T + bp

On-chip factorization (exact): exp(lrelu(s)) = exp(0.2s) * max(exp(0.8s), 1).
The per-query factor exp(0.2 a_i) cancels in softmax, so the unnormalized
weight used on-chip is
    w[j,i] = adjT[j,i] * vb02_j * max(exp(0.8 a_i + 0.8 b_j), 1)
with vb02_j = exp(0.2 b_j) folded into the value stationaries host-side.

Per key-block (128 keys x 1024 queries), per head the masked weights are
built one of two ways (to spread work across engines):
  ACT-form (heads 0,1):  e = ScalarE exp(abc + b08_j)  [per-partition bias],
                         pm = DVE stt: (e max 1) * mask
  z-form  (heads 2,3):   z = ea08b * mask   (TT mult, DVE/GPSIMD)
                         pm = (z * vb08_j) max mask    (stt, DVE/GPSIMD)
      since mask in {0,1}: max(z*vb08, mask) = mask * max(exp(.8s), 1).

All setup tensors (Wh, score rows, exp factors, scaled stationaries) are
precomputed on host; device setup is pure DMA + 8 small broadcast matmuls.
adj is host-transposed to bf16 so mask loads are plain contiguous DMAs.
"""

import os
from contextlib import ExitStack

import numpy as np
import ml_dtypes

import concourse.bacc as bacc
import concourse.bass as bass
import concourse.mybir as mybir
import concourse.tile as tile

F32 = mybir.dt.float32
BF16 = mybir.dt.bfloat16

ALU = mybir.AluOpType
AF = mybir.ActivationFunctionType

N = 8192          # nodes
IN_F = 256        # input features
H = 4             # heads
DH = 64           # head dim
NCORES = 8
QN = N // NCORES  # queries per core (1024)
KB = N // 128     # key blocks of 128 (64)
QH = 2            # 512-wide query halves

BF16_NP = ml_dtypes.bfloat16


def build_nc():
    nc = bacc.Bacc("TRN2", target_bir_lowering=False, debug=False)

    # host-precomputed tensors
    whv_d = nc.declare_dram_parameter("whv", [128, KB * H * (DH + 1)], BF16, False)
    adjt_d = nc.declare_dram_parameter("adjt", [N, QN], BF16, False)
    # heads 2,3: mask pre-scaled by vb08 = exp(0.8 b_j) host-side
    adjm_d = nc.declare_dram_parameter("adjm", [N, 2 * QN], BF16, False)
    a08_d = nc.declare_dram_parameter("a08", [4, QN], F32, False)      # all heads: 0.8*a
    ea08_d = nc.declare_dram_parameter("ea08", [2, QN], BF16, False)   # heads 2,3: exp(0.8*a)
    b08_d = nc.declare_dram_parameter("b08", [128, 4 * KB], F32, False)    # all heads
    wpt_d = nc.declare_dram_parameter("wpt", [IN_F, IN_F], F32, False)  # Wp.T
    bp_d = nc.declare_dram_parameter("bp", [IN_F], F32, False)
    out = nc.declare_dram_parameter("out", [QN, IN_F], F32, True)

    with ExitStack() as ctx:
        tc = ctx.enter_context(tile.TileContext(nc))

        persist = ctx.enter_context(tc.tile_pool(name="persist", bufs=1))
        whv = persist.tile([128, KB, H, DH + 1], BF16)
        abc = persist.tile([128, 4, QN], F32)      # broadcast 0.8*a rows, all heads
        eap23 = persist.tile([128, 2, QN], BF16)   # broadcast exp(0.8a), heads 2,3
        b08 = persist.tile([128, 4, KB], F32)
        wpt_sb = persist.tile([128, 2, IN_F], F32)
        bpb = persist.tile([128, IN_F], F32)
        ones_b = persist.tile([1, 128], BF16)
        ones_f32 = persist.tile([1, 128], F32)
        ones_f = persist.tile([1, 64], F32)

        # main-loop pools pinned before setup so slots don't alias setup tiles
        MBUFS = int(os.environ.get("GAT_BUFS", "4"))
        mloop = ctx.enter_context(tc.tile_pool(name="mloop", bufs=MBUFS))
        for _b in range(MBUFS):
            _t = mloop.tile([128, QN], BF16, tag="mt")
            nc.vector.memset(_t[0:1, 0:2], 0.0)
            _t = mloop.tile([128, 2, QN], BF16, tag="mp23")
            nc.vector.memset(_t[0:1, 0, 0:2], 0.0)
            _t = mloop.tile([128, 4, QN], BF16, tag="ee4")
            nc.vector.memset(_t[0:1, 0, 0:2], 0.0)
            _t = mloop.tile([128, 4, QN], BF16, tag="pm4")
            nc.vector.memset(_t[0:1, 0, 0:2], 0.0)

        # ---------------- setup: DMAs + row broadcasts ----------------
        nc.vector.memset(ones_b, 1.0)
        nc.vector.memset(ones_f32, 1.0)
        nc.vector.memset(ones_f, 1.0)

        nc.scalar.dma_start(b08, b08_d[:, :].rearrange("p (j k) -> p j k", j=4))
        nc.scalar.dma_start(wpt_sb, wpt_d[:, :].rearrange("(c p) w -> p c w", p=128))
        bp_ap = bp_d[:]
        nc.gpsimd.dma_start(bpb, bass.AP(tensor=bp_ap.tensor, offset=bp_ap.offset,
                                         ap=[[0, 128]] + list(bp_ap.ap)))
        # whv streamed in key-block chunks so the first main matmuls are not
        # gated on the full 4.25 MB stationary load
        whv_r = whv_d[:, :].rearrange("p (k h d) -> p k h d", k=KB, h=H)
        for wc in range(8):
            ks = slice(wc * (KB // 8), (wc + 1) * (KB // 8))
            nc.scalar.dma_start(whv[:, ks, :, :], whv_r[:, ks, :, :])

        WARMUP = int(os.environ.get("GAT_WARMUP", "16"))
        with tc.tile_pool(name="setup", bufs=1) as setup, \
             tc.tile_pool(name="spsum", bufs=4, space="PSUM") as spsum:
            a08row = setup.tile([1, 4, QN], F32)
            ea08row = setup.tile([1, 2, QN], BF16)
            nc.sync.dma_start(a08row, a08_d[:, :].rearrange("(o j) q -> o j q", o=1))
            nc.sync.dma_start(ea08row, ea08_d[:, :].rearrange("(o j) q -> o j q", o=1))
            # PE warm-up: back-to-back dummy matmuls (inputs depend only on
            # two small early DMAs) to flip HAM to 8/8 before the real MMs
            for w in range(WARMUP):
                pw = spsum.tile([128, 512], F32, tag="bc_a")
                nc.tensor.matmul(pw[:, 0:256], wpt_sb[:, 0, 0:128], wpt_sb[:, 1, :])
            # broadcast rows across 128 partitions via ones-matmuls
            for j in range(4):
                for qh in range(QH):
                    qsl = slice(qh * 512, (qh + 1) * 512)
                    pa = spsum.tile([128, 512], F32, tag="bc_a")
                    nc.tensor.matmul(pa, ones_f32, a08row[:, j, qsl])
                    nc.vector.tensor_copy(abc[:, j, qsl], pa)
            for j in range(2):
                for qh in range(QH):
                    qsl = slice(qh * 512, (qh + 1) * 512)
                    pe = spsum.tile([128, 512], F32, tag="bc_e")
                    nc.tensor.matmul(pe, ones_b, ea08row[:, j, qsl])
                    nc.scalar.copy(eap23[:, j, qsl], pe)

        # ---------------- main loop ----------------
        mpsum_cm = tc.tile_pool(name="mpsum", bufs=1, space="PSUM")
        mpsum = mpsum_cm.__enter__()
        acc = mpsum.tile([DH + 1, H, QH, 512], F32)

        # engine split: of the 128 mult pair-TT ops (2/block), TT_GPS go to
        # GPSIMD (Pool rejects max-TT), the rest to DVE.
        TT_GPS = int(os.environ.get("GAT_TT_GPS", "0"))  # per 128

        mi = 0

        def frac_hit(i, frac, tot):
            return (i * frac) // tot != ((i - 1) * frac) // tot

        def tt_engine():
            nonlocal mi
            mi += 1
            return nc.gpsimd if frac_hit(mi, TT_GPS, 128) else nc.vector

        # of 64 blocks, ACT4 use the all-ACT form (4 exps + one flat max +
        # one 4-plane mask mult); the rest use the split form (2 exps +
        # madj-masks for heads 2,3).
        ACT4 = int(os.environ.get("GAT_ACT4", "42"))  # per 64

        # software pipeline: the pm23 max (DVE) and all matmuls for block kb
        # are emitted DELAY iterations later, so the strict-FIFO DVE and PE
        # queues never head-of-line-block on a slow producer.
        DELAY = int(os.environ.get("GAT_DELAY", "2"))
        pend = []

        def finish_block(item):
            kb0, pm4_0, ee4_0, mt2_0 = item
            if ee4_0 is not None:
                # split-form deferred stage: mask-max for heads 2,3
                nc.vector.tensor_tensor(pm4_0[:, 2:4, :], ee4_0[:, 2:4, :],
                                        mt2_0, op=ALU.max)
            for hs in range(H):
                for qh in range(QH):
                    nc.tensor.matmul(acc[:, hs, qh, :], whv[:, kb0, hs, :],
                                     pm4_0[:, hs, qh * 512:(qh + 1) * 512],
                                     start=(kb0 == 0), stop=(kb0 == KB - 1))

        for kb in range(KB):
            act4 = frac_hit(kb + 1, ACT4, 64)
            mt = mloop.tile([128, QN], BF16, tag="mt")
            nc.sync.dma_start(mt, adjt_d[kb * 128:(kb + 1) * 128, :])
            mt2 = bass.AP(tensor=mt.tensor, offset=mt.offset,
                          ap=[list(mt.ap[0]), [0, 2], list(mt.ap[1])])
            mt4 = bass.AP(tensor=mt.tensor, offset=mt.offset,
                          ap=[list(mt.ap[0]), [0, 4], list(mt.ap[1])])
            ee4 = mloop.tile([128, 4, QN], BF16, tag="ee4")
            pm4 = mloop.tile([128, 4, QN], BF16, tag="pm4")

            if act4:
                for j in range(4):
                    nc.scalar.activation(ee4[:, j, :], abc[:, j, :], AF.Exp,
                                         bias=b08[:, j, kb:kb + 1], scale=1.0)
                eeflat = bass.AP(tensor=ee4.tensor, offset=ee4.offset,
                                 ap=[list(ee4.ap[0]), [1, 4 * QN]])
                nc.vector.tensor_scalar(eeflat, eeflat, 1.0, None, op0=ALU.max)
                nc.vector.tensor_tensor(pm4, ee4, mt4, op=ALU.mult)
                pend.append((kb, pm4, None, None))
            else:
                mp23 = mloop.tile([128, 2, QN], BF16, tag="mp23")
                nc.sync.dma_start(
                    mp23, adjm_d[kb * 128:(kb + 1) * 128, :].rearrange(
                        "p (j q) -> p j q", j=2))
                for j in range(2):
                    nc.scalar.activation(ee4[:, j, :], abc[:, j, :], AF.Exp,
                                         bias=b08[:, j, kb:kb + 1], scale=1.0)
                eeflat = bass.AP(tensor=ee4.tensor, offset=ee4.offset,
                                 ap=[list(ee4.ap[0]), [1, 2 * QN]])
                nc.vector.tensor_scalar(eeflat, eeflat, 1.0, None, op0=ALU.max)
                nc.vector.tensor_tensor(pm4[:, 0:2, :], ee4[:, 0:2, :], mt2,
                                        op=ALU.mult)
                # heads 2,3: q = ea * madj into ee4 slots 2:4
                nc.vector.tensor_tensor(ee4[:, 2:4, :], eap23, mp23,
                                        op=ALU.mult)
                pend.append((kb, pm4, ee4, mt2))

            if len(pend) > DELAY:
                finish_block(pend.pop(0))

        for item in pend:
            finish_block(item)

        # ---------------- tail: normalize, elu, out-proj ----------------
        tailp = ctx.enter_context(tc.tile_pool(name="tailp", bufs=1))
        denr = tailp.tile([1, H, QN], F32)
        gfin = tailp.tile([128, 2, QN], F32)
        graw = tailp.tile([128, 2, QN], F32)
        ACT_RECIP = int(os.environ.get("GAT_ACT_RECIP", "1"))
        for hs in range(H):
            for qh in range(QH):
                qsl = slice(qh * 512, (qh + 1) * 512)
                if ACT_RECIP:
                    # 1/den = square(1/sqrt(den)) on ACT (den > 0), keeping
                    # the iterative-divide off the DVE critical path
                    nc.scalar.activation(denr[:, hs, qsl], acc[DH:DH + 1, hs, qh, :],
                                         AF.Abs_reciprocal_sqrt)
                    nc.vector.tensor_mul(denr[:, hs, qsl], denr[:, hs, qsl],
                                         denr[:, hs, qsl])
                else:
                    nc.vector.reciprocal(denr[:, hs, qsl], acc[DH:DH + 1, hs, qh, :])
            nc.vector.tensor_copy(
                graw[(hs % 2) * 64:(hs % 2) * 64 + 64, hs // 2, :],
                acc[0:DH, hs, :, :].rearrange("p a b -> p (a b)"))
        mpsum_cm.__exit__(None, None, None)

        with tc.tile_pool(name="tpsum", bufs=2, space="PSUM") as tpsum:
            # normalize: broadcast 1/den across partitions via ones-matmul
            for j in range(2):
                for qh in range(QH):
                    qsl = slice(qh * 512, (qh + 1) * 512)
                    rps = tpsum.tile([128, 512], F32, tag="r_ps")
                    nc.tensor.matmul(rps[0:64, :], ones_f, denr[:, 2 * j, qsl])
                    nc.tensor.matmul(rps[64:128, :], ones_f, denr[:, 2 * j + 1, qsl])
                    nc.vector.tensor_mul(gfin[:, j, qsl], graw[:, j, qsl], rps)

            # elu(x) = relu(x) + exp(min(x, 0)) - 1
            for qh in range(QH):
                for j in range(2):
                    qsl = slice(qh * 512, (qh + 1) * 512)
                    t = tailp.tile([128, 512], F32, tag="elu_t")
                    nc.vector.tensor_scalar(t, gfin[:, j, qsl], 0.0, None,
                                            op0=ALU.min)
                    e = tailp.tile([128, 512], F32, tag="elu_e")
                    nc.scalar.activation(e, t, AF.Exp)
                    em1 = tailp.tile([128, 512], F32, tag="elu_em1")
                    nc.vector.tensor_scalar(em1, e, -1.0, None, op0=ALU.add)
                    nc.vector.scalar_tensor_tensor(gfin[:, j, qsl], gfin[:, j, qsl],
                                                   0.0, em1, op0=ALU.max, op1=ALU.add)

            for qc in range(QN // 128):
                qsl = slice(qc * 128, (qc + 1) * 128)
                po = tpsum.tile([128, IN_F], F32, tag="out_ps")
                nc.tensor.matmul(po, gfin[:, 0, qsl], wpt_sb[:, 0, :],
                                 start=True, stop=False)
                nc.tensor.matmul(po, gfin[:, 1, qsl], wpt_sb[:, 1, :],
                                 start=False, stop=True)
                fin = tailp.tile([128, IN_F], F32, tag="fin")
                nc.vector.scalar_tensor_tensor(fin, po, 0.0, bpb,
                                               op0=ALU.add, op1=ALU.add)
                nc.sync.dma_start(out[qsl, :], fin)

    nc.compile()
    return nc


_NC_CACHE = {}
LAST_RESULTS = None


def _get_nc():
    if "nc" not in _NC_CACHE:
        _NC_CACHE["nc"] = build_nc()
    return _NC_CACHE["nc"]


def kernel(h, adj, W, a1, a2, Wp, bp):
    from concourse.bass_utils import run_bass_kernel_spmd

    h = np.asarray(h, dtype=np.float32)
    adj = np.asarray(adj)
    W = np.asarray(W, dtype=np.float32)
    a1 = np.asarray(a1, dtype=np.float32)
    a2 = np.asarray(a2, dtype=np.float32)
    Wp = np.asarray(Wp, dtype=np.float32)
    bp = np.asarray(bp, dtype=np.float32)

    # ---- host precompute (O(N d^2): ~1% of kernel FLOPs) ----
    Wh = np.einsum("ni,hid->nhd", h, W).astype(np.float32)     # [N, H, DH]
    asc = np.einsum("nhd,hd->hn", Wh, a1)                      # [H, N]
    bsc = np.einsum("nhd,hd->hn", Wh, a2)                      # [H, N]
    vb02 = np.exp(0.2 * bsc)                                   # [H, N]
    vb08 = np.exp(0.8 * bsc)
    # value stationaries [128, KB, H, DH+1]: [Wh * vb02 | vb02]
    whv_f = np.concatenate(
        [Wh * vb02.T[:, :, None], vb02.T[:, :, None]], axis=2)  # [N, H, DH+1]
    whv_np = np.ascontiguousarray(
        whv_f.reshape(KB, 128, H, DH + 1).transpose(1, 0, 2, 3)
        .reshape(128, KB * H * (DH + 1)).astype(BF16_NP))
    b08_np = np.ascontiguousarray(
        (0.8 * bsc).T.reshape(KB, 128, H).transpose(1, 2, 0)
        .reshape(128, H * KB).astype(np.float32))
    wpt = np.ascontiguousarray(Wp.T)

    nc = _get_nc()
    in_maps = []
    for c in range(NCORES):
        qsl = slice(c * QN, (c + 1) * QN)
        adjt_f = adj[qsl, :].T.astype(np.float32)           # [N, QN]
        adjm = adjt_f[:, None, :] * vb08[2:4].T[:, :, None]  # [N, 2, QN]
        in_maps.append({
            "whv": whv_np,
            "adjt": adjt_f.astype(BF16_NP),
            "adjm": adjm.reshape(N, 2 * QN).astype(BF16_NP),
            "a08": np.ascontiguousarray(0.8 * asc[:, qsl]).astype(np.float32),
            "ea08": np.ascontiguousarray(np.exp(0.8 * asc[2:4, qsl])).astype(BF16_NP),
            "b08": b08_np,
            "wpt": wpt,
            "bp": bp,
        })

    res = run_bass_kernel_spmd(nc, in_maps, core_ids=list(range(NCORES)))
    global LAST_RESULTS
    LAST_RESULTS = res
    return np.concatenate([r["out"] for r in res.results], axis=0)


# revision 35
# speedup vs baseline: 1.6842x; 1.0232x over previous
"""Multi-head graph attention (GAT) Trainium2 kernel, v2.

Row-sharded across 8 NeuronCores: core i owns queries [i*1024, (i+1)*1024).

Math (per head h, with Wh = h @ W_h, a = Wh@a1, b = Wh@a2, s = a_i + b_j):
    e[i,j]  = leakyrelu(s, 0.2)
    attn    = softmax_j(where(adj>0, e, -9e15))
    out_h   = elu(attn @ Wh)
    out     = concat_h(out_h) @ Wp.T + bp

On-chip factorization (exact): exp(lrelu(s)) = exp(0.2s) * max(exp(0.8s), 1).
The per-query factor exp(0.2 a_i) cancels in softmax, so the unnormalized
weight used on-chip is
    w[j,i] = adjT[j,i] * vb02_j * max(exp(0.8 a_i + 0.8 b_j), 1)
with vb02_j = exp(0.2 b_j) folded into the value stationaries host-side.

Per key-block (128 keys x 1024 queries), per head the masked weights are
built one of two ways (to spread work across engines):
  ACT-form (heads 0,1):  e = ScalarE exp(abc + b08_j)  [per-partition bias],
                         pm = DVE stt: (e max 1) * mask
  z-form  (heads 2,3):   z = ea08b * mask   (TT mult, DVE/GPSIMD)
                         pm = (z * vb08_j) max mask    (stt, DVE/GPSIMD)
      since mask in {0,1}: max(z*vb08, mask) = mask * max(exp(.8s), 1).

All setup tensors (Wh, score rows, exp factors, scaled stationaries) are
precomputed on host; device setup is pure DMA + 8 small broadcast matmuls.
adj is host-transposed to bf16 so mask loads are plain contiguous DMAs.
"""

import os
from contextlib import ExitStack

import numpy as np
import ml_dtypes

import concourse.bacc as bacc
import concourse.bass as bass
import concourse.mybir as mybir
import concourse.tile as tile

F32 = mybir.dt.float32
BF16 = mybir.dt.bfloat16

ALU = mybir.AluOpType
AF = mybir.ActivationFunctionType

N = 8192          # nodes
IN_F = 256        # input features
H = 4             # heads
DH = 64           # head dim
NCORES = 8
QN = N // NCORES  # queries per core (1024)
KB = N // 128     # key blocks of 128 (64)
QH = 2            # 512-wide query halves

BF16_NP = ml_dtypes.bfloat16


def build_nc():
    nc = bacc.Bacc("TRN2", target_bir_lowering=False, debug=False)

    # host-precomputed tensors
    whv_d = nc.declare_dram_parameter("whv", [128, KB * H * (DH + 1)], BF16, False)
    adjt_d = nc.declare_dram_parameter("adjt", [N, QN], BF16, False)
    # heads 2,3: mask pre-scaled by vb08 = exp(0.8 b_j) host-side
    adjm_d = nc.declare_dram_parameter("adjm", [N, 2 * QN], BF16, False)
    a08_d = nc.declare_dram_parameter("a08", [4, QN], F32, False)      # all heads: 0.8*a
    ea08_d = nc.declare_dram_parameter("ea08", [2, QN], BF16, False)   # heads 2,3: exp(0.8*a)
    b08_d = nc.declare_dram_parameter("b08", [128, 4 * KB], F32, False)    # all heads
    wpt_d = nc.declare_dram_parameter("wpt", [IN_F, IN_F], F32, False)  # Wp.T
    bp_d = nc.declare_dram_parameter("bp", [IN_F], F32, False)
    out = nc.declare_dram_parameter("out", [QN, IN_F], F32, True)

    with ExitStack() as ctx:
        tc = ctx.enter_context(tile.TileContext(nc))

        persist = ctx.enter_context(tc.tile_pool(name="persist", bufs=1))
        whv = persist.tile([128, KB, H, DH + 1], BF16)
        abc = persist.tile([128, 4, QN], F32)      # broadcast 0.8*a rows, all heads
        eap23 = persist.tile([128, 2, QN], BF16)   # broadcast exp(0.8a), heads 2,3
        b08 = persist.tile([128, 4, KB], F32)
        wpt_sb = persist.tile([128, 2, IN_F], F32)
        bpb = persist.tile([128, IN_F], F32)
        ones_b = persist.tile([1, 128], BF16)
        ones_f32 = persist.tile([1, 128], F32)
        ones_f = persist.tile([1, 64], F32)

        # main-loop pools pinned before setup so slots don't alias setup tiles
        MBUFS = int(os.environ.get("GAT_BUFS", "4"))
        mloop = ctx.enter_context(tc.tile_pool(name="mloop", bufs=MBUFS))
        for _b in range(MBUFS):
            _t = mloop.tile([128, QN], BF16, tag="mt")
            nc.vector.memset(_t[0:1, 0:2], 0.0)
            _t = mloop.tile([128, 2, QN], BF16, tag="mp23")
            nc.vector.memset(_t[0:1, 0, 0:2], 0.0)
            _t = mloop.tile([128, 4, QN], BF16, tag="ee4")
            nc.vector.memset(_t[0:1, 0, 0:2], 0.0)
            _t = mloop.tile([128, 4, QN], BF16, tag="pm4")
            nc.vector.memset(_t[0:1, 0, 0:2], 0.0)

        # ---------------- setup: DMAs + row broadcasts ----------------
        nc.vector.memset(ones_b, 1.0)
        nc.vector.memset(ones_f32, 1.0)
        nc.vector.memset(ones_f, 1.0)

        nc.scalar.dma_start(b08, b08_d[:, :].rearrange("p (j k) -> p j k", j=4))
        nc.scalar.dma_start(wpt_sb, wpt_d[:, :].rearrange("(c p) w -> p c w", p=128))
        bp_ap = bp_d[:]
        nc.gpsimd.dma_start(bpb, bass.AP(tensor=bp_ap.tensor, offset=bp_ap.offset,
                                         ap=[[0, 128]] + list(bp_ap.ap)))
        # whv streamed in key-block chunks so the first main matmuls are not
        # gated on the full 4.25 MB stationary load
        whv_r = whv_d[:, :].rearrange("p (k h d) -> p k h d", k=KB, h=H)
        for wc in range(8):
            ks = slice(wc * (KB // 8), (wc + 1) * (KB // 8))
            nc.scalar.dma_start(whv[:, ks, :, :], whv_r[:, ks, :, :])

        WARMUP = int(os.environ.get("GAT_WARMUP", "16"))
        with tc.tile_pool(name="setup", bufs=1) as setup, \
             tc.tile_pool(name="spsum", bufs=4, space="PSUM") as spsum:
            a08row = setup.tile([1, 4, QN], F32)
            ea08row = setup.tile([1, 2, QN], BF16)
            nc.sync.dma_start(a08row, a08_d[:, :].rearrange("(o j) q -> o j q", o=1))
            nc.sync.dma_start(ea08row, ea08_d[:, :].rearrange("(o j) q -> o j q", o=1))
            # broadcast rows across 128 partitions via ones-matmuls
            for j in range(4):
                for qh in range(QH):
                    qsl = slice(qh * 512, (qh + 1) * 512)
                    pa = spsum.tile([128, 512], F32, tag="bc_a")
                    nc.tensor.matmul(pa, ones_f32, a08row[:, j, qsl])
                    nc.vector.tensor_copy(abc[:, j, qsl], pa)
            for j in range(2):
                for qh in range(QH):
                    qsl = slice(qh * 512, (qh + 1) * 512)
                    pe = spsum.tile([128, 512], F32, tag="bc_e")
                    nc.tensor.matmul(pe, ones_b, ea08row[:, j, qsl])
                    nc.scalar.copy(eap23[:, j, qsl], pe)
            # PE warm-up: back-to-back dummy matmuls to flip HAM to 8/8
            # before the real MMs
            for w in range(WARMUP):
                pw = spsum.tile([128, 512], F32, tag="bc_a")
                nc.tensor.matmul(pw[:, 0:256], wpt_sb[:, 0, 0:128], wpt_sb[:, 1, :])

        # ---------------- main loop ----------------
        mpsum_cm = tc.tile_pool(name="mpsum", bufs=1, space="PSUM")
        mpsum = mpsum_cm.__enter__()
        acc = mpsum.tile([DH + 1, H, QH, 512], F32)

        # engine split: of the 128 mult pair-TT ops (2/block), TT_GPS go to
        # GPSIMD (Pool rejects max-TT), the rest to DVE.
        TT_GPS = int(os.environ.get("GAT_TT_GPS", "0"))  # per 128

        mi = 0

        def frac_hit(i, frac, tot):
            return (i * frac) // tot != ((i - 1) * frac) // tot

        def tt_engine():
            nonlocal mi
            mi += 1
            return nc.gpsimd if frac_hit(mi, TT_GPS, 128) else nc.vector

        # of 64 blocks, ACT4 use the all-ACT form (4 exps + one flat max +
        # one 4-plane mask mult); the rest use the split form (2 exps +
        # madj-masks for heads 2,3).
        ACT4 = int(os.environ.get("GAT_ACT4", "42"))  # per 64

        # software pipeline: the pm23 max (DVE) and all matmuls for block kb
        # are emitted DELAY iterations later, so the strict-FIFO DVE and PE
        # queues never head-of-line-block on a slow producer.
        DELAY = int(os.environ.get("GAT_DELAY", "2"))
        pend = []

        def finish_block(item):
            kb0, pm4_0, ee4_0, mt2_0 = item
            if ee4_0 is not None:
                # split-form deferred stage: mask-max for heads 2,3
                nc.vector.tensor_tensor(pm4_0[:, 2:4, :], ee4_0[:, 2:4, :],
                                        mt2_0, op=ALU.max)
            for hs in range(H):
                for qh in range(QH):
                    nc.tensor.matmul(acc[:, hs, qh, :], whv[:, kb0, hs, :],
                                     pm4_0[:, hs, qh * 512:(qh + 1) * 512],
                                     start=(kb0 == 0), stop=(kb0 == KB - 1))

        for kb in range(KB):
            act4 = frac_hit(kb + 1, ACT4, 64)
            mt = mloop.tile([128, QN], BF16, tag="mt")
            nc.sync.dma_start(mt, adjt_d[kb * 128:(kb + 1) * 128, :])
            mt2 = bass.AP(tensor=mt.tensor, offset=mt.offset,
                          ap=[list(mt.ap[0]), [0, 2], list(mt.ap[1])])
            mt4 = bass.AP(tensor=mt.tensor, offset=mt.offset,
                          ap=[list(mt.ap[0]), [0, 4], list(mt.ap[1])])
            ee4 = mloop.tile([128, 4, QN], BF16, tag="ee4")
            pm4 = mloop.tile([128, 4, QN], BF16, tag="pm4")

            if act4:
                for j in range(4):
                    nc.scalar.activation(ee4[:, j, :], abc[:, j, :], AF.Exp,
                                         bias=b08[:, j, kb:kb + 1], scale=1.0)
                eeflat = bass.AP(tensor=ee4.tensor, offset=ee4.offset,
                                 ap=[list(ee4.ap[0]), [1, 4 * QN]])
                nc.vector.tensor_scalar(eeflat, eeflat, 1.0, None, op0=ALU.max)
                nc.vector.tensor_tensor(pm4, ee4, mt4, op=ALU.mult)
                pend.append((kb, pm4, None, None))
            else:
                mp23 = mloop.tile([128, 2, QN], BF16, tag="mp23")
                nc.sync.dma_start(
                    mp23, adjm_d[kb * 128:(kb + 1) * 128, :].rearrange(
                        "p (j q) -> p j q", j=2))
                for j in range(2):
                    nc.scalar.activation(ee4[:, j, :], abc[:, j, :], AF.Exp,
                                         bias=b08[:, j, kb:kb + 1], scale=1.0)
                eeflat = bass.AP(tensor=ee4.tensor, offset=ee4.offset,
                                 ap=[list(ee4.ap[0]), [1, 2 * QN]])
                nc.vector.tensor_scalar(eeflat, eeflat, 1.0, None, op0=ALU.max)
                nc.vector.tensor_tensor(pm4[:, 0:2, :], ee4[:, 0:2, :], mt2,
                                        op=ALU.mult)
                # heads 2,3: q = ea * madj into ee4 slots 2:4
                nc.vector.tensor_tensor(ee4[:, 2:4, :], eap23, mp23,
                                        op=ALU.mult)
                pend.append((kb, pm4, ee4, mt2))

            if len(pend) > DELAY:
                finish_block(pend.pop(0))

        for item in pend:
            finish_block(item)

        # ---------------- tail: normalize, elu, out-proj ----------------
        tailp = ctx.enter_context(tc.tile_pool(name="tailp", bufs=1))
        denr = tailp.tile([1, H, QN], F32)
        gfin = tailp.tile([128, 2, QN], F32)
        graw = tailp.tile([128, 2, QN], F32)
        ACT_RECIP = int(os.environ.get("GAT_ACT_RECIP", "1"))
        for hs in range(H):
            for qh in range(QH):
                qsl = slice(qh * 512, (qh + 1) * 512)
                if ACT_RECIP:
                    # 1/den = square(1/sqrt(den)) on ACT (den > 0), keeping
                    # the iterative-divide off the DVE critical path
                    nc.scalar.activation(denr[:, hs, qsl], acc[DH:DH + 1, hs, qh, :],
                                         AF.Abs_reciprocal_sqrt)
                    nc.vector.tensor_mul(denr[:, hs, qsl], denr[:, hs, qsl],
                                         denr[:, hs, qsl])
                else:
                    nc.vector.reciprocal(denr[:, hs, qsl], acc[DH:DH + 1, hs, qh, :])
            nc.vector.tensor_copy(
                graw[(hs % 2) * 64:(hs % 2) * 64 + 64, hs // 2, :],
                acc[0:DH, hs, :, :].rearrange("p a b -> p (a b)"))
        mpsum_cm.__exit__(None, None, None)

        with tc.tile_pool(name="tpsum", bufs=2, space="PSUM") as tpsum:
            # normalize: broadcast 1/den across partitions via ones-matmul
            for j in range(2):
                for qh in range(QH):
                    qsl = slice(qh * 512, (qh + 1) * 512)
                    rps = tpsum.tile([128, 512], F32, tag="r_ps")
                    nc.tensor.matmul(rps[0:64, :], ones_f, denr[:, 2 * j, qsl])
                    nc.tensor.matmul(rps[64:128, :], ones_f, denr[:, 2 * j + 1, qsl])
                    nc.vector.tensor_mul(gfin[:, j, qsl], graw[:, j, qsl], rps)

            # elu(x) = relu(x) + exp(min(x, 0)) - 1
            for qh in range(QH):
                for j in range(2):
                    qsl = slice(qh * 512, (qh + 1) * 512)
                    t = tailp.tile([128, 512], F32, tag="elu_t")
                    nc.vector.tensor_scalar(t, gfin[:, j, qsl], 0.0, None,
                                            op0=ALU.min)
                    e = tailp.tile([128, 512], F32, tag="elu_e")
                    nc.scalar.activation(e, t, AF.Exp)
                    em1 = tailp.tile([128, 512], F32, tag="elu_em1")
                    nc.vector.tensor_scalar(em1, e, -1.0, None, op0=ALU.add)
                    nc.vector.scalar_tensor_tensor(gfin[:, j, qsl], gfin[:, j, qsl],
                                                   0.0, em1, op0=ALU.max, op1=ALU.add)

            for qc in range(QN // 128):
                qsl = slice(qc * 128, (qc + 1) * 128)
                po = tpsum.tile([128, IN_F], F32, tag="out_ps")
                nc.tensor.matmul(po, gfin[:, 0, qsl], wpt_sb[:, 0, :],
                                 start=True, stop=False)
                nc.tensor.matmul(po, gfin[:, 1, qsl], wpt_sb[:, 1, :],
                                 start=False, stop=True)
                fin = tailp.tile([128, IN_F], F32, tag="fin")
                nc.vector.scalar_tensor_tensor(fin, po, 0.0, bpb,
                                               op0=ALU.add, op1=ALU.add)
                nc.sync.dma_start(out[qsl, :], fin)

    nc.compile()
    return nc


_NC_CACHE = {}
LAST_RESULTS = None


def _get_nc():
    if "nc" not in _NC_CACHE:
        _NC_CACHE["nc"] = build_nc()
    return _NC_CACHE["nc"]


def kernel(h, adj, W, a1, a2, Wp, bp):
    from concourse.bass_utils import run_bass_kernel_spmd

    h = np.asarray(h, dtype=np.float32)
    adj = np.asarray(adj)
    W = np.asarray(W, dtype=np.float32)
    a1 = np.asarray(a1, dtype=np.float32)
    a2 = np.asarray(a2, dtype=np.float32)
    Wp = np.asarray(Wp, dtype=np.float32)
    bp = np.asarray(bp, dtype=np.float32)

    # ---- host precompute (O(N d^2): ~1% of kernel FLOPs) ----
    Wh = np.einsum("ni,hid->nhd", h, W).astype(np.float32)     # [N, H, DH]
    asc = np.einsum("nhd,hd->hn", Wh, a1)                      # [H, N]
    bsc = np.einsum("nhd,hd->hn", Wh, a2)                      # [H, N]
    vb02 = np.exp(0.2 * bsc)                                   # [H, N]
    vb08 = np.exp(0.8 * bsc)
    # value stationaries [128, KB, H, DH+1]: [Wh * vb02 | vb02]
    whv_f = np.concatenate(
        [Wh * vb02.T[:, :, None], vb02.T[:, :, None]], axis=2)  # [N, H, DH+1]
    whv_np = np.ascontiguousarray(
        whv_f.reshape(KB, 128, H, DH + 1).transpose(1, 0, 2, 3)
        .reshape(128, KB * H * (DH + 1)).astype(BF16_NP))
    b08_np = np.ascontiguousarray(
        (0.8 * bsc).T.reshape(KB, 128, H).transpose(1, 2, 0)
        .reshape(128, H * KB).astype(np.float32))
    wpt = np.ascontiguousarray(Wp.T)

    nc = _get_nc()
    in_maps = []
    for c in range(NCORES):
        qsl = slice(c * QN, (c + 1) * QN)
        adjt_f = adj[qsl, :].T.astype(np.float32)           # [N, QN]
        adjm = adjt_f[:, None, :] * vb08[2:4].T[:, :, None]  # [N, 2, QN]
        in_maps.append({
            "whv": whv_np,
            "adjt": adjt_f.astype(BF16_NP),
            "adjm": adjm.reshape(N, 2 * QN).astype(BF16_NP),
            "a08": np.ascontiguousarray(0.8 * asc[:, qsl]).astype(np.float32),
            "ea08": np.ascontiguousarray(np.exp(0.8 * asc[2:4, qsl])).astype(BF16_NP),
            "b08": b08_np,
            "wpt": wpt,
            "bp": bp,
        })

    res = run_bass_kernel_spmd(nc, in_maps, core_ids=list(range(NCORES)))
    global LAST_RESULTS
    LAST_RESULTS = res
    return np.concatenate([r["out"] for r in res.results], axis=0)
